# revision 19
# baseline (speedup 1.0000x reference)
"""Trainium2 Bass kernel for nn_FCGF_point_att3_sft_7000 (8 NeuronCores).

Model: pointwise attention MLP (32->16->8->1, BN+relu, BN stats over the full
512000-point batch), per-segment softmax over 2000 points, attention-weighted
pooling to [256, 64000], FC head 64000->1024->256 (BN+relu, stats over the
256-segment batch), final L2 row-normalize.

Sharding: points-within-segment. Core c owns points p in [250c, 250(c+1)) of
every segment. Stage A is data-parallel over points with AllGather'd BN stats;
fc1 is contraction-sharded (each core owns 8000 of the 64000 inputs and the
matching fw1 rows), summed via ReduceScatter whose per-shard aux row also
carries the softmax denominators; fc2 is contraction-sharded and finished with
an AllReduce; the tail is replicated.

Stage-A layout: "quartered" A-orientation. x.T is [128, 16000] with the
channels of free-quarter a on partitions [32a, 32a+32). Matmuls use
tile_position=(32a, 32a) so outputs land on partitions 32a+ch and every
eviction / BN / softmax op runs 128 partitions wide. Weight tiles are
zero-padded to M=32 so all PSUM rows are defined.

Training-mode BN is shift-invariant => conv/linear biases (b1,b2,b3,fb1,fb2)
drop out exactly; they are accepted and ignored.
"""

import sys

sys.path.insert(0, "/opt/trn_rl_repo")

import numpy as np

import concourse.bass as bass
import concourse.tile as tile
from concourse import mybir
from concourse.masks import make_identity

B = 256
P = 2000
C = 32
NCORES = 8
PL = P // NCORES           # 250
PH = PL // 2               # 125
NPTS = B * PL              # 64000 points per core
QF = NPTS // 4             # 16000 per quarter
NCH = 500                  # stage-A free chunk
NCHUNK = QF // NCH         # 32
EPS_BN = 1e-5
F32 = mybir.dt.float32
BF16 = mybir.dt.float16  # fp16: same speed as bf16, 8x lower rounding noise
RG = [list(range(NCORES))]
AF = mybir.ActivationFunctionType

_cache = {}


# ------------------------------------------------------------------ walrus fix
def _install_walrus_patch():
    """This container's walrus accepts only ONE semaphore wait per instruction.
    Spread Tile's end-of-kernel drain waits across single-wait nops, and split
    any instruction carrying >1 waits onto same-engine carrier nops."""
    if _cache.get("patched"):
        return
    from concourse.vector_clock import ScopedClock, VectorClock

    counter = [0]

    def split_waits(nc):
        for bb in nc.main_func.blocks:
            out = []
            changed = False
            for ins in bb.instructions:
                si = ins.sync_info
                waits = list(si.on_wait) if si and si.on_wait else []
                if len(waits) > 1:
                    changed = True
                    for w in waits[:-1]:
                        counter[0] += 1
                        out.append(mybir.InstNoOp(
                            name=f"I-wsplit-{counter[0]}",
                            engine=ins.engine, ins=[], outs=[],
                            sync_info=mybir.SyncInfo(on_wait=[w], on_update=[]),
                            bass_nofuse=True))
                    si.on_wait = waits[-1:]
                out.append(ins)
            if changed:
                try:
                    bb.instructions = out
                except Exception:
                    bb.instructions.clear()
                    for x in out:
                        bb.instructions.append(x)

    def _patched(self, tick_clock, wait_clock):
        nc = self.nc
        gc = tick_clock.global_clock
        n = len(gc)
        for i in range(n):
            if gc[i] > 0:
                vec = [0] * n
                vec[i] = gc[i]
                nop = nc.sync.nop(nofuse=True, hint=f"drain_wait_p{i}")
                wait_clock.add_sem_waits(
                    nop.ins, ScopedClock({None: VectorClock(vec)}))
        nc.sync.drain()
        nc.all_engine_barrier()
        assert self.sems is not None
        popped = nc._tile_sem_poison_stack.pop()
        assert popped is self._sem_poison
        nc.clear_and_free_semaphores(list(self.sems.allocated().values()))
        nc.all_engine_barrier()
        split_waits(nc)

    tile.TileContext._drain_and_barrier = _patched
    _cache["patched"] = True


# ------------------------------------------------------------------ bass build
def _build():
    _install_walrus_patch()
    nc = bass.Bass()

    def ein(name, shape, dt):
        return nc.dram_tensor(name, shape, dt, kind="ExternalInput")

    d = {}
    d["xA4"] = ein("xA4", [128, QF], BF16)
    d["xB"] = ein("xB", [PH, C * 2 * B], BF16)
    d["w1D"] = ein("w1D", [128, 128], BF16)
    d["w2D"] = ein("w2D", [128, 128], BF16)
    d["w3D"] = ein("w3D", [128, 128], BF16)
    for n in ("g1q", "be1q", "g2q", "be2q"):
        d[n] = ein(n, [128, 1], F32)
    d["g3s"] = ein("g3s", [1, 1], F32)
    d["be3s"] = ein("be3s", [1, 1], F32)
    d["f1"] = ein("f1", [128, 16], F32)
    d["ft1"] = ein("ft1", [16, 128], F32)
    d["f2"] = ein("f2", [128, 8], F32)
    d["ft2"] = ein("ft2", [8, 128], F32)
    d["f8_16"] = ein("f8_16", [128, 16], F32)
    d["f8_8"] = ein("f8_8", [64, 8], F32)
    d["fw1t"] = ein("fw1t", [PH, C * 2, 1024], BF16)
    d["fw2t"] = ein("fw2t", [128, 256], BF16)
    d["fg1s"] = ein("fg1s", [128, 1], F32)
    d["fbe1s"] = ein("fbe1s", [128, 1], F32)
    d["fg2t"] = ein("fg2t", [128, 2], F32)
    d["fbe2t"] = ein("fbe2t", [128, 2], F32)
    d["out_final"] = nc.dram_tensor("out_final", [256, 256], F32,
                                    kind="ExternalOutput")
    # collective bounce buffers
    d["warm_i"] = nc.dram_tensor("warm_i", [16, 4], F32)
    d["warm_o"] = nc.dram_tensor("warm_o", [16, 4], F32)
    d["st1_i"] = nc.dram_tensor("st1_i", [16, 2], F32)
    d["st1_o"] = nc.dram_tensor("st1_o", [128, 2], F32)
    d["st2_i"] = nc.dram_tensor("st2_i", [8, 2], F32)
    d["st2_o"] = nc.dram_tensor("st2_o", [64, 2], F32)
    d["st3_i"] = nc.dram_tensor("st3_i", [1, 2], F32)
    d["st3_o"] = nc.dram_tensor("st3_o", [8, 2], F32)
    d["rs5_i"] = nc.dram_tensor("rs5_i", [NCORES * 129, 256], F32)
    d["rs5_o"] = nc.dram_tensor("rs5_o", [129, 256], F32)
    d["ar6_i"] = nc.dram_tensor("ar6_i", [256, 256], F32)
    d["ar6_o"] = nc.dram_tensor("ar6_o", [256, 256], F32)

    with tile.TileContext(nc) as tc:
        _body(nc, tc, d)
    return nc


def _mkstats(nc, pool, mv, count, name):
    """mv [p,2]=(mean,var) -> (sum,sumsq) [p,2]."""
    p = mv.shape[0]
    ss = pool.tile([p, 2], F32, tag=f"ss_{name}")
    nc.vector.tensor_mul(ss[:, 1:2], mv[:, 0:1], mv[:, 0:1])
    nc.vector.tensor_add(ss[:, 1:2], ss[:, 1:2], mv[:, 1:2])
    nc.scalar.mul(ss[:, 0:1], mv[:, 0:1], float(count))
    nc.scalar.mul(ss[:, 1:2], ss[:, 1:2], float(count))
    return ss


def _mv_from_ss(nc, pool, ss, count, name):
    """(sum,sumsq) [p,2] over count -> (mean, rstd) [p,2]."""
    p = ss.shape[0]
    mr = pool.tile([p, 2], F32, tag=f"mr_{name}")
    epst = pool.tile([p, 1], F32, tag=f"eps_{name}")
    nc.vector.memset(epst[:], EPS_BN)
    nc.scalar.mul(mr[:, 0:1], ss[:, 0:1], 1.0 / count)
    nc.scalar.mul(mr[:, 1:2], ss[:, 1:2], 1.0 / count)
    m2 = pool.tile([p, 1], F32, tag=f"m2_{name}")
    nc.vector.tensor_mul(m2[:], mr[:, 0:1], mr[:, 0:1])
    nc.vector.tensor_sub(mr[:, 1:2], mr[:, 1:2], m2[:])
    nc.scalar.activation(mr[:, 1:2], mr[:, 1:2], AF.Sqrt, bias=epst[:])
    nc.vector.reciprocal(mr[:, 1:2], mr[:, 1:2])
    return mr


def _scale_bias(nc, pool, mrq, g, be, name):
    """scale = g*rstd ; bias = be - scale*mean  (all [p,1] per-partition)."""
    p = mrq.shape[0]
    sc = pool.tile([p, 1], F32, tag=f"sc_{name}")
    bi = pool.tile([p, 1], F32, tag=f"bi_{name}")
    nc.vector.tensor_mul(sc[:], g[:], mrq[:, 1:2])
    nc.vector.tensor_mul(bi[:], sc[:], mrq[:, 0:1])
    nc.vector.tensor_sub(bi[:], be[:], bi[:])
    return sc, bi


def _body(nc, tc, d):
    # collective warmup first — input copied dram->dram (no engine deps), so
    # the ~55us ncfw startup overlaps the whole front of the kernel.
    nc.gpsimd.dma_start(d["warm_i"][:], d["f1"][0:16, 0:4])
    nc.gpsimd.collective_compute(
        "AllReduce", mybir.AluOpType.add, replica_groups=RG,
        ins=[d["warm_i"][:]], outs=[d["warm_o"][:]])

    sing_cm = tc.tile_pool(name="sing", bufs=1)
    big_cm = tc.tile_pool(name="big", bufs=1)
    work_cm = tc.tile_pool(name="work", bufs=1)
    psA_cm = tc.tile_pool(name="psA", bufs=4, space="PSUM")
    psT_cm = tc.tile_pool(name="psT", bufs=2, space="PSUM")
    psS_cm = tc.tile_pool(name="psS", bufs=2, space="PSUM")
    sing = sing_cm.__enter__(); big = big_cm.__enter__()
    work = work_cm.__enter__()
    fw1p_cm = tc.tile_pool(name="fw1p", bufs=2)
    fw1p = fw1p_cm.__enter__()
    psA = psA_cm.__enter__(); psT = psT_cm.__enter__()
    psS = psS_cm.__enter__()

    # ---------------- constants
    def load(name, shape, dt=F32, pool=sing):
        t = pool.tile(shape, dt, tag=name)
        nc.sync.dma_start(t[:], d[name][:])
        return t

    w1D = load("w1D", [128, 128], BF16)
    w2D = load("w2D", [128, 128], BF16)
    w3D = load("w3D", [128, 128], BF16)
    f1s = load("f1", [128, 16])
    ft1s = load("ft1", [16, 128])
    f2s = load("f2", [128, 8])
    ft2s = load("ft2", [8, 128])
    f8_16s = load("f8_16", [128, 16])
    f8_8s = load("f8_8", [64, 8])
    g1 = load("g1q", [128, 1]); be1 = load("be1q", [128, 1])
    g2 = load("g2q", [128, 1]); be2 = load("be2q", [128, 1])
    g3 = load("g3s", [1, 1]); be3 = load("be3s", [1, 1])
    ones128 = sing.tile([128, 1], F32)
    nc.vector.memset(ones128[:], 1.0)
    ones8 = sing.tile([8, 1], F32)
    nc.vector.memset(ones8[:], 1.0)
    ones1x = sing.tile([1, 128], F32)
    nc.vector.memset(ones1x[:], 1.0)
    ident = sing.tile([128, 128], F32)
    make_identity(nc, ident[:])

    # ---------------- big loads
    xa = big.tile([128, QF], BF16, tag="slotA")       # slot A: xa -> y2 -> y3q
    nc.sync.dma_start(xa[:], d["xA4"][:])
    xb = big.tile([PH, C * 2 * B], BF16, tag="xb")
    nc.sync.dma_start(xb[:], d["xB"][:])
    xbv = xb[:].rearrange("p (c h s) -> p c h s", c=C, h=2, s=B)

    # fc1 weight prefetch: pool entered at the top so its slots exist from
    # t=0 and the 16.4MB stream overlaps all of stage A. 3-engine rotation.
    ITS_PER_DMA = 8
    fwview = d["fw1t"][:].rearrange("p (g i) o -> p g i o", i=ITS_PER_DMA)
    fwtiles = []
    _dge = [nc.sync, nc.scalar]
    for gblk in range(C * 2 // ITS_PER_DMA):
        fwt = fw1p.tile([PH, ITS_PER_DMA, 1024], BF16, tag="fw",
                        name=f"fw_{gblk}")
        _dge[gblk % 2].dma_start(fwt[:], fwview[:, gblk, :, :])
        fwtiles.append(fwt)

    def layer_mms(ps, wD, krows, rhs_src, sl):
        nc.tensor.matmul(ps[:], wD[:], rhs_src[:, sl], start=True, stop=True)

    def stage_layer(rhs_src, wT, krows, fold, foldT, f8fold, st_i, st_o,
                    gq, beq, count_local, name, out_tag):
        """Single-pass layer: matmuls -> evict y f32 (+bn_stats from PSUM),
        fold+AllGather stats, then BN+relu applied in place (h aliases y)."""
        y = big.tile([128, QF], BF16, tag=out_tag, name=f"y_{name}")
        stat = work.tile([128, NCHUNK, 6], F32, tag=f"stat_{name}")
        for j in range(NCHUNK):
            ps = psA.tile([128, NCH], F32, tag="psA", name=f"ps_{name}_{j}")
            layer_mms(ps, wT, krows, rhs_src, slice(j * NCH, (j + 1) * NCH))
            nc.scalar.copy(y[:, j * NCH : (j + 1) * NCH], ps[:])
            nc.vector.bn_stats(stat[:, j, :], ps[:])
        mv = work.tile([128, 2], F32, tag=f"mv_{name}")
        nc.vector.bn_aggr(mv[:], stat[:])
        ss = _mkstats(nc, work, mv, count_local, name)
        nfold = fold.shape[1]
        psf = psS.tile([128, 2], F32, tag="small", name=f"psf_{name}")
        nc.tensor.matmul(psf[:nfold, :], fold[:], ss[:], start=True, stop=True)
        sbf = work.tile([nfold, 2], F32, tag=f"sbf_{name}")
        nc.scalar.copy(sbf[:], psf[:nfold, :])
        nc.gpsimd.dma_start(st_i[:], sbf[:])
        nc.gpsimd.collective_compute(
            "AllGather", mybir.AluOpType.bypass, replica_groups=RG,
            ins=[st_i[:]], outs=[st_o[:]])
        agg = work.tile([nfold * NCORES, 2], F32, tag=f"agg_{name}")
        nc.gpsimd.dma_start(agg[:], st_o[:])
        psg = psS.tile([128, 2], F32, tag="small", name=f"psg_{name}")
        nc.tensor.matmul(psg[:nfold, :], f8fold[:], agg[:], start=True,
                         stop=True)
        ssg = work.tile([nfold, 2], F32, tag=f"ssg_{name}")
        nc.scalar.copy(ssg[:], psg[:nfold, :])
        mr = _mv_from_ss(nc, work, ssg, B * P, name)
        psb = psS.tile([128, 2], F32, tag="small", name=f"psb_{name}")
        nc.tensor.matmul(psb[:], foldT[:], mr[:], start=True, stop=True)
        mrq = work.tile([128, 2], F32, tag=f"mrq_{name}")
        nc.scalar.copy(mrq[:], psb[:])
        sc, bi = _scale_bias(nc, work, mrq, gq, beq, name)
        for j in range(NCHUNK):
            sl = slice(j * NCH, (j + 1) * NCH)
            nc.scalar.activation(y[:, sl], y[:, sl], AF.Relu,
                                 bias=bi[:], scale=sc[:])
        return y

    # ---------------- stage A layers 1 & 2
    h1 = stage_layer(xa, w1D, 32, f1s, ft1s, f8_16s,
                     d["st1_i"], d["st1_o"], g1, be1, QF, "l1", "slotB")
    # h2 reuses slot A (xa dead after L1 matmuls)
    h2 = stage_layer(h1, w2D, 16, f2s, ft2s, f8_8s,
                     d["st2_i"], d["st2_o"], g2, be2, QF, "l2", "slotA")

    # ---------------- stage A layer 3: scores straight from PSUM into
    # scoreS [128 segs, 2, 250] via per-chunk repack DMAs (rows {32a} real;
    # chunk j of quarter a covers segments 64a+2j..+1)
    scoreS = big.tile([128, 2, PL], F32, tag="scoreS")
    for j in range(NCHUNK):
        ps = psA.tile([128, NCH], F32, tag="psA", name=f"ps_l3_{j}")
        layer_mms(ps, w3D, 8, h2, slice(j * NCH, (j + 1) * NCH))
        y3t = work.tile([128, NCH], F32, tag="y3t", name=f"y3t_{j}", bufs=3)
        nc.scalar.copy(y3t[:], ps[:])
        for a in range(4):
            eng = nc.sync if (j * 4 + a) % 2 == 0 else nc.scalar
            eng.dma_start(
                scoreS[64 * (a % 2) + 2 * j : 64 * (a % 2) + 2 * j + 2,
                       a // 2, :],
                y3t[32 * a : 32 * a + 1, :])
    # BN3 stats over all segments/points (all partitions real)
    stat3 = work.tile([128, 2, 6], F32, tag="stat3")
    nc.vector.bn_stats(stat3[:, 0, :], scoreS[:, 0, :])
    nc.vector.bn_stats(stat3[:, 1, :], scoreS[:, 1, :])
    mv3 = work.tile([128, 2], F32, tag="mv3")
    nc.vector.bn_aggr(mv3[:], stat3[:])
    ss3 = _mkstats(nc, work, mv3, 2 * PL, "l3")
    psf3 = psS.tile([128, 2], F32, tag="small", name="psf3")
    nc.tensor.matmul(psf3[:1, :], ones128[:], ss3[:], start=True, stop=True)
    sbf3 = work.tile([1, 2], F32, tag="sbf3")
    nc.scalar.copy(sbf3[:], psf3[:1, :])
    nc.gpsimd.dma_start(d["st3_i"][:], sbf3[:])
    nc.gpsimd.collective_compute(
        "AllGather", mybir.AluOpType.bypass, replica_groups=RG,
        ins=[d["st3_i"][:]], outs=[d["st3_o"][:]])
    agg3 = work.tile([8, 2], F32, tag="agg3")
    nc.gpsimd.dma_start(agg3[:], d["st3_o"][:])
    psg3 = psS.tile([128, 2], F32, tag="small", name="psg3")
    nc.tensor.matmul(psg3[:1, :], ones8[:], agg3[:], start=True, stop=True)
    ssg3 = work.tile([1, 2], F32, tag="ssg3")
    nc.scalar.copy(ssg3[:], psg3[:1, :])
    mr3 = _mv_from_ss(nc, work, ssg3, B * P, "l3")
    scb1 = work.tile([1, 2], F32, tag="scb1")
    nc.vector.tensor_mul(scb1[:, 0:1], g3[:], mr3[:, 1:2])
    nc.vector.tensor_mul(scb1[:, 1:2], scb1[:, 0:1], mr3[:, 0:1])
    nc.vector.tensor_sub(scb1[:, 1:2], be3[:], scb1[:, 1:2])
    psb3 = psS.tile([128, 2], F32, tag="small", name="psb3")
    nc.tensor.matmul(psb3[:], ones1x[:], scb1[:], start=True, stop=True)
    scb = work.tile([128, 2], F32, tag="scb")
    nc.scalar.copy(scb[:], psb3[:])
    # relu(BN3) in place, then exp
    expS = big.tile([128, 2, PL], F32, tag="expS")
    for tt in range(2):
        nc.scalar.activation(scoreS[:, tt, :], scoreS[:, tt, :], AF.Relu,
                             bias=scb[:, 1:2], scale=scb[:, 0:1])
        nc.scalar.activation(expS[:, tt, :], scoreS[:, tt, :], AF.Exp)
        # partial softmax denominators
    zloc = work.tile([128, 2], F32, tag="zloc")
    nc.vector.reduce_sum(zloc[:, 0:1], expS[:, 0, :], axis=mybir.AxisListType.X)
    nc.vector.reduce_sum(zloc[:, 1:2], expS[:, 1, :], axis=mybir.AxisListType.X)
    # z into every shard's aux row of rs5_i (8 small DMAs; dst col = 128*tt+s)
    for cc in range(NCORES):
        dst = d["rs5_i"][cc * 129 + 128 : cc * 129 + 129, :].rearrange(
            "r (t s) -> r s t", t=2, s=128)
        nc.sync.dma_start(dst, zloc[:])
    # expT [125, 2, 256]: PE-transpose expS halves
    expT = big.tile([PH, 2, 256], F32, tag="expT")
    for h in range(2):
        for tt in range(2):
            pt_ps = psT.tile([128, 128], F32, tag="psT")
            nc.tensor.transpose(pt_ps[:PH, :],
                                expS[:, tt, h * PH : h * PH + PH], ident[:])
            nc.scalar.copy(expT[:, h, tt * 128 : tt * 128 + 128],
                           pt_ps[:PH, :])

    psS_cm.__exit__(None, None, None)
    psT_cm.__exit__(None, None, None)
    psA_cm.__exit__(None, None, None)

    # ---------------- FC1 (contraction-sharded, out [1024, 256] partial)
    psF_cm = tc.tile_pool(name="psF", bufs=1, space="PSUM")
    ptp_cm = tc.tile_pool(name="ptp", bufs=3)
    psF = psF_cm.__enter__()
    ptp = ptp_cm.__enter__()
    r1ps = [psF.tile([128, 256], F32, name=f"r1ps_{m}", tag=f"r1_{m}")
            for m in range(8)]
    NIT = C * 2
    for ch in range(C):
        for h in range(2):
            it = ch * 2 + h
            fw = fwtiles[it // ITS_PER_DMA][:, it % ITS_PER_DMA, :]
            pt = ptp.tile([PH, 256], BF16, tag="pt", name=f"pt_{it}")
            nc.vector.tensor_mul(pt[:], xbv[:, ch, h, :], expT[:, h, :])
            for m in range(8):
                nc.tensor.matmul(
                    r1ps[m][:, :], fw[:, m * 128 : (m + 1) * 128], pt[:],
                    start=(it == 0), stop=(it == NIT - 1))
    for m in range(8):
        r1sb = big.tile([128, 256], F32, tag="r1sb", name=f"r1sb_{m}", bufs=2)
        nc.scalar.copy(r1sb[:], r1ps[m][:])
        nc.sync.dma_start(d["rs5_i"][m * 129 : m * 129 + 128, :], r1sb[:])
    nc.gpsimd.collective_compute(
        "ReduceScatter", mybir.AluOpType.add, replica_groups=RG,
        ins=[d["rs5_i"][:]], outs=[d["rs5_o"][:]])

    ptp_cm.__exit__(None, None, None)
    psF_cm.__exit__(None, None, None)
    fw1p_cm.__exit__(None, None, None)

    # ---------------- FC1 finish + FC2 + tail
    ps2_cm = tc.tile_pool(name="ps2", bufs=1, space="PSUM")
    ps2 = ps2_cm.__enter__()

    r1 = big.tile([128, 256], F32, tag="r1")
    nc.sync.dma_start(r1[:], d["rs5_o"][0:128, :])
    zrow = work.tile([1, 256], F32, tag="zrow")
    nc.sync.dma_start(zrow[:], d["rs5_o"][128:129, :])
    nc.vector.reciprocal(zrow[:], zrow[:])
    ps_z = ps2.tile([128, 256], F32, tag="zb")
    nc.tensor.matmul(ps_z[:], ones1x[:], zrow[:], start=True, stop=True)
    zinv = big.tile([128, 256], F32, tag="zinv")
    nc.scalar.copy(zinv[:], ps_z[:])
    nc.vector.tensor_mul(r1[:], r1[:], zinv[:])
    # BN over segments (free dim), relu
    stf1 = work.tile([128, 6], F32, tag="stf1")
    nc.vector.bn_stats(stf1[:], r1[:])
    mvf1 = work.tile([128, 2], F32, tag="mvf1")
    nc.vector.bn_aggr(mvf1[:], stf1[:])
    epsf = work.tile([128, 1], F32, tag="epsf")
    nc.vector.memset(epsf[:], EPS_BN)
    nc.scalar.activation(mvf1[:, 1:2], mvf1[:, 1:2], AF.Sqrt, bias=epsf[:])
    nc.vector.reciprocal(mvf1[:, 1:2], mvf1[:, 1:2])
    fg1 = load("fg1s", [128, 1], pool=work)
    fbe1 = load("fbe1s", [128, 1], pool=work)
    scf1, bif1 = _scale_bias(nc, work, mvf1, fg1, fbe1, "f1")
    r1b = big.tile([128, 256], BF16, tag="r1b")
    nc.scalar.activation(r1b[:], r1[:], AF.Relu, bias=bif1[:], scale=scf1[:])
    # FC2 partial
    fw2 = load("fw2t", [128, 256], BF16, pool=work)
    r2sb = big.tile([128, 2, 256], F32, tag="r2sb")
    for m in range(2):
        ps_r2 = ps2.tile([128, 256], F32, tag=f"r2_{m}")
        nc.tensor.matmul(ps_r2[:], fw2[:, m * 128 : (m + 1) * 128], r1b[:],
                         start=True, stop=True)
        nc.scalar.copy(r2sb[:, m, :], ps_r2[:])
        nc.sync.dma_start(d["ar6_i"][m * 128 : (m + 1) * 128, :],
                          r2sb[:, m, :])
    nc.gpsimd.collective_compute(
        "AllReduce", mybir.AluOpType.add, replica_groups=RG,
        ins=[d["ar6_i"][:]], outs=[d["ar6_o"][:]])

    # tail: BN over segments per o2-row, relu, transpose, L2-normalize
    fg2 = load("fg2t", [128, 2], pool=work)
    fbe2 = load("fbe2t", [128, 2], pool=work)
    outT = big.tile([128, 2, 256], F32, tag="outT")
    for m in range(2):
        r2 = big.tile([128, 256], F32, tag="r2")
        nc.sync.dma_start(r2[:], d["ar6_o"][m * 128 : (m + 1) * 128, :])
        stf2 = work.tile([128, 6], F32, tag="stf2")
        nc.vector.bn_stats(stf2[:], r2[:])
        mvf2 = work.tile([128, 2], F32, tag="mvf2")
        nc.vector.bn_aggr(mvf2[:], stf2[:])
        nc.scalar.activation(mvf2[:, 1:2], mvf2[:, 1:2], AF.Sqrt, bias=epsf[:])
        nc.vector.reciprocal(mvf2[:, 1:2], mvf2[:, 1:2])
        scf2, bif2 = _scale_bias(nc, work, mvf2,
                                 fg2[:, m : m + 1], fbe2[:, m : m + 1], "f2")
        nc.scalar.activation(r2[:], r2[:], AF.Relu, bias=bif2[:], scale=scf2[:])
        for tt in range(2):
            ps_t = ps2.tile([128, 128], F32, tag="tailT")
            nc.tensor.transpose(ps_t[:], r2[:, tt * 128 : (tt + 1) * 128],
                                ident[:])
            nc.scalar.copy(outT[:, tt, m * 128 : (m + 1) * 128], ps_t[:])
    for tt in range(2):
        sq = big.tile([128, 256], F32, tag="sq")
        nc.scalar.activation(sq[:], outT[:, tt, :], AF.Square)
        nrm = work.tile([128, 1], F32, tag="nrm")
        nc.vector.reduce_sum(nrm[:], sq[:], axis=mybir.AxisListType.X)
        nc.scalar.activation(nrm[:], nrm[:], AF.Sqrt)
        nc.vector.tensor_scalar_max(nrm[:], nrm[:], 1e-12)
        nc.vector.reciprocal(nrm[:], nrm[:])
        nc.vector.tensor_scalar_mul(outT[:, tt, :], outT[:, tt, :], nrm[:])
        nc.sync.dma_start(d["out_final"][tt * 128 : (tt + 1) * 128, :],
                          outT[:, tt, :])

    ps2_cm.__exit__(None, None, None)
    work_cm.__exit__(None, None, None)
    big_cm.__exit__(None, None, None)
    sing_cm.__exit__(None, None, None)


# ------------------------------------------------------------------ host side
def _prep_core(x3, fw1, c):
    import ml_dtypes
    xs = x3[:, PL * c : PL * (c + 1), :]                       # [256,250,32]
    arr = np.ascontiguousarray(xs.transpose(2, 0, 1))          # [32,256,250]
    xA4 = arr.reshape(C, 4, QF).transpose(1, 0, 2).reshape(128, QF)
    xb = xs.reshape(B, 2, PH, C).transpose(2, 3, 1, 0)         # [125,32,2,256]
    xB = np.ascontiguousarray(xb).reshape(PH, C * 2 * B)
    fw = fw1.reshape(1024, P, C)[:, PL * c : PL * (c + 1), :]
    fw = fw.reshape(1024, 2, PH, C).transpose(2, 3, 1, 0)      # [125,32,2,1024]
    fw1t = np.ascontiguousarray(fw).reshape(PH, C * 2, 1024)
    bf = np.float16
    return (np.ascontiguousarray(xA4).astype(bf), xB.astype(bf),
            fw1t.astype(bf))


def _qrep(v, rows):
    out = np.zeros((128, 1), np.float32)
    for a in range(4):
        out[32 * a : 32 * a + rows, 0] = v
    return out


def _wdiag(w):
    """w [out,in] -> block-diagonal lhsT [128, 128]: block a (32x32) holds
    w.T in its top-left corner."""
    t = np.zeros((128, 128), np.float32)
    wt = w.T  # [in, out]
    for a in range(4):
        t[32 * a : 32 * a + wt.shape[0], 32 * a : 32 * a + wt.shape[1]] = wt
    return t


def kernel(**inputs):
    import ml_dtypes

    if "nc" not in _cache:
        _cache["nc"] = _build()
    nc = _cache["nc"]
    bf = np.float16

    g = {k: np.asarray(v, np.float32) for k, v in inputs.items()
         if k != "length"}
    x3 = g["x"].reshape(B, P, C)

    f1 = np.zeros((128, 16), np.float32)
    f2 = np.zeros((128, 8), np.float32)
    for a in range(4):
        f1[32 * a : 32 * a + 16, :] = np.eye(16, dtype=np.float32)
        f2[32 * a : 32 * a + 8, :] = np.eye(8, dtype=np.float32)
    f8_16 = np.zeros((128, 16), np.float32)
    f8_8 = np.zeros((64, 8), np.float32)
    for k in range(8):
        f8_16[16 * k : 16 * k + 16, :] = np.eye(16, dtype=np.float32)
        f8_8[8 * k : 8 * k + 8, :] = np.eye(8, dtype=np.float32)

    shared = {
        "w1D": _wdiag(g["w1"]).astype(bf),
        "w2D": _wdiag(g["w2"]).astype(bf),
        "w3D": _wdiag(g["w3"]).astype(bf),
        "g1q": _qrep(g["g1"], 16), "be1q": _qrep(g["be1"], 16),
        "g2q": _qrep(g["g2"], 8), "be2q": _qrep(g["be2"], 8),
        "g3s": g["g3"].reshape(1, 1), "be3s": g["be3"].reshape(1, 1),
        "f1": f1, "ft1": np.ascontiguousarray(f1.T),
        "f2": f2, "ft2": np.ascontiguousarray(f2.T),
        "f8_16": f8_16, "f8_8": f8_8,
        "fg2t": np.ascontiguousarray(g["fg2"].reshape(2, 128).T),
        "fbe2t": np.ascontiguousarray(g["fbe2"].reshape(2, 128).T),
    }

    in_maps = []
    for c in range(NCORES):
        xA4, xB, fw1t = _prep_core(x3, g["fw1"], c)
        m = dict(shared)
        m["xA4"] = xA4
        m["xB"] = xB
        m["fw1t"] = fw1t
        m["fw2t"] = np.ascontiguousarray(
            g["fw2"][:, 128 * c : 128 * (c + 1)].T).astype(bf)
        m["fg1s"] = g["fg1"][128 * c : 128 * (c + 1)].reshape(128, 1)
        m["fbe1s"] = g["fbe1"][128 * c : 128 * (c + 1)].reshape(128, 1)
        in_maps.append(m)

    from concourse.bass_utils import run_bass_kernel_spmd

    res = run_bass_kernel_spmd(nc, in_maps, core_ids=list(range(NCORES)),
                               trace=bool(_cache.get("trace")))
    _cache["last_result"] = res
    return np.asarray(res.results[0]["out_final"], np.float32)


if __name__ == "__main__":
    nc = _build()
    print("build ok; instructions:",
          sum(len(bb.instructions) for bb in nc.main_func.blocks))


# revision 20
# speedup vs baseline: 1.2027x; 1.2027x over previous
"""Trainium2 Bass kernel for nn_FCGF_point_att3_sft_7000 (8 NeuronCores).

Model: pointwise attention MLP (32->16->8->1, BN+relu, BN stats over the full
512000-point batch), per-segment softmax over 2000 points, attention-weighted
pooling to [256, 64000], FC head 64000->1024->256 (BN+relu, stats over the
256-segment batch), final L2 row-normalize.

Sharding: points-within-segment. Core c owns points p in [250c, 250(c+1)) of
every segment. Stage A is data-parallel over points with AllGather'd BN stats;
fc1 is contraction-sharded (each core owns 8000 of the 64000 inputs and the
matching fw1 rows), summed via ReduceScatter whose per-shard aux row also
carries the softmax denominators; fc2 is contraction-sharded and finished with
an AllReduce; the tail is replicated.

Stage-A layout: "quartered" A-orientation. x.T is [128, 16000] with the
channels of free-quarter a on partitions [32a, 32a+32). Matmuls use
tile_position=(32a, 32a) so outputs land on partitions 32a+ch and every
eviction / BN / softmax op runs 128 partitions wide. Weight tiles are
zero-padded to M=32 so all PSUM rows are defined.

Training-mode BN is shift-invariant => conv/linear biases (b1,b2,b3,fb1,fb2)
drop out exactly; they are accepted and ignored.
"""

import sys

sys.path.insert(0, "/opt/trn_rl_repo")

import numpy as np

import concourse.bass as bass
import concourse.tile as tile
from concourse import mybir
from concourse.masks import make_identity

B = 256
P = 2000
C = 32
NCORES = 8
PL = P // NCORES           # 250
PH = PL // 2               # 125
NPTS = B * PL              # 64000 points per core
QF = NPTS // 4             # 16000 per quarter
NCH = 500                  # stage-A free chunk
NCHUNK = QF // NCH         # 32
EPS_BN = 1e-5
F32 = mybir.dt.float32
BF16 = mybir.dt.float16  # fp16: same speed as bf16, 8x lower rounding noise
RG = [list(range(NCORES))]
AF = mybir.ActivationFunctionType

_cache = {}


# ------------------------------------------------------------------ walrus fix
def _install_walrus_patch():
    """This container's walrus accepts only ONE semaphore wait per instruction.
    Spread Tile's end-of-kernel drain waits across single-wait nops, and split
    any instruction carrying >1 waits onto same-engine carrier nops."""
    if _cache.get("patched"):
        return
    from concourse.vector_clock import ScopedClock, VectorClock

    counter = [0]

    def split_waits(nc):
        for bb in nc.main_func.blocks:
            out = []
            changed = False
            for ins in bb.instructions:
                si = ins.sync_info
                waits = list(si.on_wait) if si and si.on_wait else []
                if len(waits) > 1:
                    changed = True
                    for w in waits[:-1]:
                        counter[0] += 1
                        out.append(mybir.InstNoOp(
                            name=f"I-wsplit-{counter[0]}",
                            engine=ins.engine, ins=[], outs=[],
                            sync_info=mybir.SyncInfo(on_wait=[w], on_update=[]),
                            bass_nofuse=True))
                    si.on_wait = waits[-1:]
                out.append(ins)
            if changed:
                try:
                    bb.instructions = out
                except Exception:
                    bb.instructions.clear()
                    for x in out:
                        bb.instructions.append(x)

    def _patched(self, tick_clock, wait_clock):
        nc = self.nc
        gc = tick_clock.global_clock
        n = len(gc)
        for i in range(n):
            if gc[i] > 0:
                vec = [0] * n
                vec[i] = gc[i]
                nop = nc.sync.nop(nofuse=True, hint=f"drain_wait_p{i}")
                wait_clock.add_sem_waits(
                    nop.ins, ScopedClock({None: VectorClock(vec)}))
        nc.sync.drain()
        nc.all_engine_barrier()
        assert self.sems is not None
        popped = nc._tile_sem_poison_stack.pop()
        assert popped is self._sem_poison
        nc.clear_and_free_semaphores(list(self.sems.allocated().values()))
        nc.all_engine_barrier()
        split_waits(nc)

    tile.TileContext._drain_and_barrier = _patched
    _cache["patched"] = True


# ------------------------------------------------------------------ bass build
def _build():
    _install_walrus_patch()
    nc = bass.Bass()

    def ein(name, shape, dt):
        return nc.dram_tensor(name, shape, dt, kind="ExternalInput")

    d = {}
    d["xA4"] = ein("xA4", [128, QF], BF16)
    d["xB"] = ein("xB", [PH, C * 2 * B], BF16)
    d["w1D"] = ein("w1D", [128, 128], BF16)
    d["w2D"] = ein("w2D", [128, 128], BF16)
    d["w3D"] = ein("w3D", [128, 128], BF16)
    for n in ("g1q", "be1q", "g2q", "be2q"):
        d[n] = ein(n, [128, 1], F32)
    d["g3s"] = ein("g3s", [1, 1], F32)
    d["be3s"] = ein("be3s", [1, 1], F32)
    d["f1"] = ein("f1", [128, 16], F32)
    d["ft1"] = ein("ft1", [16, 128], F32)
    d["f2"] = ein("f2", [128, 8], F32)
    d["ft2"] = ein("ft2", [8, 128], F32)
    d["f8_16"] = ein("f8_16", [128, 16], F32)
    d["f8_8"] = ein("f8_8", [64, 8], F32)
    d["fw1t"] = ein("fw1t", [PH, C * 2, 1024], BF16)
    d["fw2t"] = ein("fw2t", [128, 256], BF16)
    d["fg1s"] = ein("fg1s", [128, 1], F32)
    d["fbe1s"] = ein("fbe1s", [128, 1], F32)
    d["fg2t"] = ein("fg2t", [128, 2], F32)
    d["fbe2t"] = ein("fbe2t", [128, 2], F32)
    d["out_final"] = nc.dram_tensor("out_final", [256, 256], F32,
                                    kind="ExternalOutput")
    # collective bounce buffers
    d["warm_i"] = nc.dram_tensor("warm_i", [16, 4], F32)
    d["warm_o"] = nc.dram_tensor("warm_o", [16, 4], F32)
    d["st1_i"] = nc.dram_tensor("st1_i", [16, 2], F32)
    d["st1_o"] = nc.dram_tensor("st1_o", [128, 2], F32)
    d["st2_i"] = nc.dram_tensor("st2_i", [8, 2], F32)
    d["st2_o"] = nc.dram_tensor("st2_o", [64, 2], F32)
    d["st3_i"] = nc.dram_tensor("st3_i", [1, 2], F32)
    d["st3_o"] = nc.dram_tensor("st3_o", [8, 2], F32)
    d["rs5_i"] = nc.dram_tensor("rs5_i", [NCORES * 129, 256], F32)
    d["rs5_o"] = nc.dram_tensor("rs5_o", [129, 256], F32)
    d["ar6_i"] = nc.dram_tensor("ar6_i", [256, 256], F32)
    d["ar6_o"] = nc.dram_tensor("ar6_o", [256, 256], F32)

    with tile.TileContext(nc) as tc:
        _body(nc, tc, d)
    return nc


def _mkstats(nc, pool, mv, count, name):
    """mv [p,2]=(mean,var) -> (sum,sumsq) [p,2]."""
    p = mv.shape[0]
    ss = pool.tile([p, 2], F32, tag=f"ss_{name}")
    nc.vector.tensor_mul(ss[:, 1:2], mv[:, 0:1], mv[:, 0:1])
    nc.vector.tensor_add(ss[:, 1:2], ss[:, 1:2], mv[:, 1:2])
    nc.scalar.mul(ss[:, 0:1], mv[:, 0:1], float(count))
    nc.scalar.mul(ss[:, 1:2], ss[:, 1:2], float(count))
    return ss


def _mv_from_ss(nc, pool, ss, count, name):
    """(sum,sumsq) [p,2] over count -> (mean, rstd) [p,2]."""
    p = ss.shape[0]
    mr = pool.tile([p, 2], F32, tag=f"mr_{name}")
    epst = pool.tile([p, 1], F32, tag=f"eps_{name}")
    nc.vector.memset(epst[:], EPS_BN)
    nc.scalar.mul(mr[:, 0:1], ss[:, 0:1], 1.0 / count)
    nc.scalar.mul(mr[:, 1:2], ss[:, 1:2], 1.0 / count)
    m2 = pool.tile([p, 1], F32, tag=f"m2_{name}")
    nc.vector.tensor_mul(m2[:], mr[:, 0:1], mr[:, 0:1])
    nc.vector.tensor_sub(mr[:, 1:2], mr[:, 1:2], m2[:])
    nc.scalar.activation(mr[:, 1:2], mr[:, 1:2], AF.Sqrt, bias=epst[:])
    nc.vector.reciprocal(mr[:, 1:2], mr[:, 1:2])
    return mr


def _scale_bias(nc, pool, mrq, g, be, name):
    """scale = g*rstd ; bias = be - scale*mean  (all [p,1] per-partition)."""
    p = mrq.shape[0]
    sc = pool.tile([p, 1], F32, tag=f"sc_{name}")
    bi = pool.tile([p, 1], F32, tag=f"bi_{name}")
    nc.vector.tensor_mul(sc[:], g[:], mrq[:, 1:2])
    nc.vector.tensor_mul(bi[:], sc[:], mrq[:, 0:1])
    nc.vector.tensor_sub(bi[:], be[:], bi[:])
    return sc, bi


def _body(nc, tc, d):
    # collective warmup first — input copied dram->dram (no engine deps), so
    # the ~55us ncfw startup overlaps the whole front of the kernel.
    nc.gpsimd.dma_start(d["warm_i"][:], d["f1"][0:16, 0:4])
    nc.gpsimd.collective_compute(
        "AllReduce", mybir.AluOpType.add, replica_groups=RG,
        ins=[d["warm_i"][:]], outs=[d["warm_o"][:]])

    sing_cm = tc.tile_pool(name="sing", bufs=1)
    big_cm = tc.tile_pool(name="big", bufs=1)
    work_cm = tc.tile_pool(name="work", bufs=1)
    psA_cm = tc.tile_pool(name="psA", bufs=4, space="PSUM")
    psT_cm = tc.tile_pool(name="psT", bufs=2, space="PSUM")
    psS_cm = tc.tile_pool(name="psS", bufs=2, space="PSUM")
    sing = sing_cm.__enter__(); big = big_cm.__enter__()
    work = work_cm.__enter__()
    fw1p_cm = tc.tile_pool(name="fw1p", bufs=2)
    fw1p = fw1p_cm.__enter__()
    psA = psA_cm.__enter__(); psT = psT_cm.__enter__()
    psS = psS_cm.__enter__()

    # ---------------- constants
    def load(name, shape, dt=F32, pool=sing):
        t = pool.tile(shape, dt, tag=name)
        nc.sync.dma_start(t[:], d[name][:])
        return t

    w1D = load("w1D", [128, 128], BF16)
    w2D = load("w2D", [128, 128], BF16)
    w3D = load("w3D", [128, 128], BF16)
    f1s = load("f1", [128, 16])
    ft1s = load("ft1", [16, 128])
    f2s = load("f2", [128, 8])
    ft2s = load("ft2", [8, 128])
    f8_16s = load("f8_16", [128, 16])
    f8_8s = load("f8_8", [64, 8])
    g1 = load("g1q", [128, 1]); be1 = load("be1q", [128, 1])
    g2 = load("g2q", [128, 1]); be2 = load("be2q", [128, 1])
    g3 = load("g3s", [1, 1]); be3 = load("be3s", [1, 1])
    ones128 = sing.tile([128, 1], F32)
    nc.vector.memset(ones128[:], 1.0)
    ones8 = sing.tile([8, 1], F32)
    nc.vector.memset(ones8[:], 1.0)
    ones1x = sing.tile([1, 128], F32)
    nc.vector.memset(ones1x[:], 1.0)
    ident = sing.tile([128, 128], F32)
    make_identity(nc, ident[:])

    # ---------------- big loads
    xa = big.tile([128, QF], BF16, tag="slotA")       # slot A: xa -> y2 -> y3q
    nc.sync.dma_start(xa[:], d["xA4"][:])
    xb = big.tile([PH, C * 2 * B], BF16, tag="xb")
    nc.sync.dma_start(xb[:], d["xB"][:])
    xbv = xb[:].rearrange("p (c h s) -> p c h s", c=C, h=2, s=B)

    # fc1 weight prefetch: pool entered at the top so its slots exist from
    # t=0 and the 16.4MB stream overlaps all of stage A. 3-engine rotation.
    ITS_PER_DMA = 8
    fwview = d["fw1t"][:].rearrange("p (g i) o -> p g i o", i=ITS_PER_DMA)
    fwtiles = []
    _dge = [nc.sync, nc.scalar, nc.gpsimd]
    for gblk in range(C * 2 // ITS_PER_DMA):
        fwt = fw1p.tile([PH, ITS_PER_DMA, 1024], BF16, tag="fw",
                        name=f"fw_{gblk}")
        _dge[gblk % 3].dma_start(fwt[:], fwview[:, gblk, :, :])
        fwtiles.append(fwt)

    def layer_mms(ps, wD, krows, rhs_src, sl):
        nc.tensor.matmul(ps[:], wD[:], rhs_src[:, sl], start=True, stop=True)

    def stage_layer(rhs_src, wT, krows, fold, foldT, f8fold, st_i, st_o,
                    gq, beq, count_local, name, out_tag):
        """Single-pass layer: matmuls -> evict y f32 (+bn_stats from PSUM),
        fold+AllGather stats, then BN+relu applied in place (h aliases y)."""
        y = big.tile([128, QF], BF16, tag=out_tag, name=f"y_{name}")
        stat = work.tile([128, NCHUNK, 6], F32, tag=f"stat_{name}")
        for j in range(NCHUNK):
            ps = psA.tile([128, NCH], F32, tag="psA", name=f"ps_{name}_{j}")
            layer_mms(ps, wT, krows, rhs_src, slice(j * NCH, (j + 1) * NCH))
            nc.scalar.copy(y[:, j * NCH : (j + 1) * NCH], ps[:])
            nc.vector.bn_stats(stat[:, j, :], ps[:])
        mv = work.tile([128, 2], F32, tag=f"mv_{name}")
        nc.vector.bn_aggr(mv[:], stat[:])
        ss = _mkstats(nc, work, mv, count_local, name)
        nfold = fold.shape[1]
        psf = psS.tile([128, 2], F32, tag="small", name=f"psf_{name}")
        nc.tensor.matmul(psf[:nfold, :], fold[:], ss[:], start=True, stop=True)
        sbf = work.tile([nfold, 2], F32, tag=f"sbf_{name}")
        nc.scalar.copy(sbf[:], psf[:nfold, :])
        nc.gpsimd.dma_start(st_i[:], sbf[:])
        nc.gpsimd.collective_compute(
            "AllGather", mybir.AluOpType.bypass, replica_groups=RG,
            ins=[st_i[:]], outs=[st_o[:]])
        agg = work.tile([nfold * NCORES, 2], F32, tag=f"agg_{name}")
        nc.gpsimd.dma_start(agg[:], st_o[:])
        psg = psS.tile([128, 2], F32, tag="small", name=f"psg_{name}")
        nc.tensor.matmul(psg[:nfold, :], f8fold[:], agg[:], start=True,
                         stop=True)
        ssg = work.tile([nfold, 2], F32, tag=f"ssg_{name}")
        nc.scalar.copy(ssg[:], psg[:nfold, :])
        mr = _mv_from_ss(nc, work, ssg, B * P, name)
        psb = psS.tile([128, 2], F32, tag="small", name=f"psb_{name}")
        nc.tensor.matmul(psb[:], foldT[:], mr[:], start=True, stop=True)
        mrq = work.tile([128, 2], F32, tag=f"mrq_{name}")
        nc.scalar.copy(mrq[:], psb[:])
        sc, bi = _scale_bias(nc, work, mrq, gq, beq, name)
        for j in range(NCHUNK):
            sl = slice(j * NCH, (j + 1) * NCH)
            nc.scalar.activation(y[:, sl], y[:, sl], AF.Relu,
                                 bias=bi[:], scale=sc[:])
        return y

    # ---------------- stage A layers 1 & 2
    h1 = stage_layer(xa, w1D, 32, f1s, ft1s, f8_16s,
                     d["st1_i"], d["st1_o"], g1, be1, QF, "l1", "slotB")
    # h2 reuses slot A (xa dead after L1 matmuls)
    h2 = stage_layer(h1, w2D, 16, f2s, ft2s, f8_8s,
                     d["st2_i"], d["st2_o"], g2, be2, QF, "l2", "slotA")

    # ---------------- stage A layer 3: scores straight from PSUM into
    # scoreS [128 segs, 2, 250] via per-chunk repack DMAs (rows {32a} real;
    # chunk j of quarter a covers segments 64a+2j..+1)
    scoreS = big.tile([128, 2, PL], F32, tag="scoreS")
    for j in range(NCHUNK):
        ps = psA.tile([128, NCH], F32, tag="psA", name=f"ps_l3_{j}")
        layer_mms(ps, w3D, 8, h2, slice(j * NCH, (j + 1) * NCH))
        y3t = work.tile([128, NCH], F32, tag="y3t", name=f"y3t_{j}", bufs=3)
        nc.scalar.copy(y3t[:], ps[:])
        for a in range(4):
            nc.sync.dma_start(
                scoreS[64 * (a % 2) + 2 * j : 64 * (a % 2) + 2 * j + 2,
                       a // 2, :],
                y3t[32 * a : 32 * a + 1, :])
    # BN3 stats over all segments/points (all partitions real)
    stat3 = work.tile([128, 2, 6], F32, tag="stat3")
    nc.vector.bn_stats(stat3[:, 0, :], scoreS[:, 0, :])
    nc.vector.bn_stats(stat3[:, 1, :], scoreS[:, 1, :])
    mv3 = work.tile([128, 2], F32, tag="mv3")
    nc.vector.bn_aggr(mv3[:], stat3[:])
    ss3 = _mkstats(nc, work, mv3, 2 * PL, "l3")
    psf3 = psS.tile([128, 2], F32, tag="small", name="psf3")
    nc.tensor.matmul(psf3[:1, :], ones128[:], ss3[:], start=True, stop=True)
    sbf3 = work.tile([1, 2], F32, tag="sbf3")
    nc.scalar.copy(sbf3[:], psf3[:1, :])
    nc.gpsimd.dma_start(d["st3_i"][:], sbf3[:])
    nc.gpsimd.collective_compute(
        "AllGather", mybir.AluOpType.bypass, replica_groups=RG,
        ins=[d["st3_i"][:]], outs=[d["st3_o"][:]])
    agg3 = work.tile([8, 2], F32, tag="agg3")
    nc.gpsimd.dma_start(agg3[:], d["st3_o"][:])
    psg3 = psS.tile([128, 2], F32, tag="small", name="psg3")
    nc.tensor.matmul(psg3[:1, :], ones8[:], agg3[:], start=True, stop=True)
    ssg3 = work.tile([1, 2], F32, tag="ssg3")
    nc.scalar.copy(ssg3[:], psg3[:1, :])
    mr3 = _mv_from_ss(nc, work, ssg3, B * P, "l3")
    scb1 = work.tile([1, 2], F32, tag="scb1")
    nc.vector.tensor_mul(scb1[:, 0:1], g3[:], mr3[:, 1:2])
    nc.vector.tensor_mul(scb1[:, 1:2], scb1[:, 0:1], mr3[:, 0:1])
    nc.vector.tensor_sub(scb1[:, 1:2], be3[:], scb1[:, 1:2])
    psb3 = psS.tile([128, 2], F32, tag="small", name="psb3")
    nc.tensor.matmul(psb3[:], ones1x[:], scb1[:], start=True, stop=True)
    scb = work.tile([128, 2], F32, tag="scb")
    nc.scalar.copy(scb[:], psb3[:])
    # relu(BN3) in place, then exp
    expS = big.tile([128, 2, PL], F32, tag="expS")
    for tt in range(2):
        nc.scalar.activation(scoreS[:, tt, :], scoreS[:, tt, :], AF.Relu,
                             bias=scb[:, 1:2], scale=scb[:, 0:1])
        nc.scalar.activation(expS[:, tt, :], scoreS[:, tt, :], AF.Exp)
        # partial softmax denominators
    zloc = work.tile([128, 2], F32, tag="zloc")
    nc.vector.reduce_sum(zloc[:, 0:1], expS[:, 0, :], axis=mybir.AxisListType.X)
    nc.vector.reduce_sum(zloc[:, 1:2], expS[:, 1, :], axis=mybir.AxisListType.X)
    # z into every shard's aux row of rs5_i (8 small DMAs; dst col = 128*tt+s)
    for cc in range(NCORES):
        dst = d["rs5_i"][cc * 129 + 128 : cc * 129 + 129, :].rearrange(
            "r (t s) -> r s t", t=2, s=128)
        nc.sync.dma_start(dst, zloc[:])
    # expT [125, 2, 256]: PE-transpose expS halves
    expT = big.tile([PH, 2, 256], F32, tag="expT")
    for h in range(2):
        for tt in range(2):
            pt_ps = psT.tile([128, 128], F32, tag="psT")
            nc.tensor.transpose(pt_ps[:PH, :],
                                expS[:, tt, h * PH : h * PH + PH], ident[:])
            nc.scalar.copy(expT[:, h, tt * 128 : tt * 128 + 128],
                           pt_ps[:PH, :])

    psS_cm.__exit__(None, None, None)
    psT_cm.__exit__(None, None, None)
    psA_cm.__exit__(None, None, None)

    # ---------------- FC1 (contraction-sharded, out [1024, 256] partial)
    psF_cm = tc.tile_pool(name="psF", bufs=1, space="PSUM")
    ptp_cm = tc.tile_pool(name="ptp", bufs=3)
    psF = psF_cm.__enter__()
    ptp = ptp_cm.__enter__()
    r1ps = [psF.tile([128, 256], F32, name=f"r1ps_{m}", tag=f"r1_{m}")
            for m in range(8)]
    NIT = C * 2
    for ch in range(C):
        for h in range(2):
            it = ch * 2 + h
            fw = fwtiles[it // ITS_PER_DMA][:, it % ITS_PER_DMA, :]
            pt = ptp.tile([PH, 256], BF16, tag="pt", name=f"pt_{it}")
            nc.vector.tensor_mul(pt[:], xbv[:, ch, h, :], expT[:, h, :])
            for m in range(8):
                nc.tensor.matmul(
                    r1ps[m][:, :], fw[:, m * 128 : (m + 1) * 128], pt[:],
                    start=(it == 0), stop=(it == NIT - 1))
    for m in range(8):
        r1sb = big.tile([128, 256], F32, tag="r1sb", name=f"r1sb_{m}", bufs=2)
        nc.scalar.copy(r1sb[:], r1ps[m][:])
        nc.sync.dma_start(d["rs5_i"][m * 129 : m * 129 + 128, :], r1sb[:])
    nc.gpsimd.collective_compute(
        "ReduceScatter", mybir.AluOpType.add, replica_groups=RG,
        ins=[d["rs5_i"][:]], outs=[d["rs5_o"][:]])

    ptp_cm.__exit__(None, None, None)
    psF_cm.__exit__(None, None, None)
    fw1p_cm.__exit__(None, None, None)

    # ---------------- FC1 finish + FC2 + tail
    ps2_cm = tc.tile_pool(name="ps2", bufs=1, space="PSUM")
    ps2 = ps2_cm.__enter__()

    r1 = big.tile([128, 256], F32, tag="r1")
    nc.sync.dma_start(r1[:], d["rs5_o"][0:128, :])
    zrow = work.tile([1, 256], F32, tag="zrow")
    nc.sync.dma_start(zrow[:], d["rs5_o"][128:129, :])
    nc.vector.reciprocal(zrow[:], zrow[:])
    ps_z = ps2.tile([128, 256], F32, tag="zb")
    nc.tensor.matmul(ps_z[:], ones1x[:], zrow[:], start=True, stop=True)
    zinv = big.tile([128, 256], F32, tag="zinv")
    nc.scalar.copy(zinv[:], ps_z[:])
    nc.vector.tensor_mul(r1[:], r1[:], zinv[:])
    # BN over segments (free dim), relu
    stf1 = work.tile([128, 6], F32, tag="stf1")
    nc.vector.bn_stats(stf1[:], r1[:])
    mvf1 = work.tile([128, 2], F32, tag="mvf1")
    nc.vector.bn_aggr(mvf1[:], stf1[:])
    epsf = work.tile([128, 1], F32, tag="epsf")
    nc.vector.memset(epsf[:], EPS_BN)
    nc.scalar.activation(mvf1[:, 1:2], mvf1[:, 1:2], AF.Sqrt, bias=epsf[:])
    nc.vector.reciprocal(mvf1[:, 1:2], mvf1[:, 1:2])
    fg1 = load("fg1s", [128, 1], pool=work)
    fbe1 = load("fbe1s", [128, 1], pool=work)
    scf1, bif1 = _scale_bias(nc, work, mvf1, fg1, fbe1, "f1")
    r1b = big.tile([128, 256], BF16, tag="r1b")
    nc.scalar.activation(r1b[:], r1[:], AF.Relu, bias=bif1[:], scale=scf1[:])
    # FC2 partial
    fw2 = load("fw2t", [128, 256], BF16, pool=work)
    r2sb = big.tile([128, 2, 256], F32, tag="r2sb")
    for m in range(2):
        ps_r2 = ps2.tile([128, 256], F32, tag=f"r2_{m}")
        nc.tensor.matmul(ps_r2[:], fw2[:, m * 128 : (m + 1) * 128], r1b[:],
                         start=True, stop=True)
        nc.scalar.copy(r2sb[:, m, :], ps_r2[:])
        nc.sync.dma_start(d["ar6_i"][m * 128 : (m + 1) * 128, :],
                          r2sb[:, m, :])
    nc.gpsimd.collective_compute(
        "AllReduce", mybir.AluOpType.add, replica_groups=RG,
        ins=[d["ar6_i"][:]], outs=[d["ar6_o"][:]])

    # tail: BN over segments per o2-row, relu, transpose, L2-normalize
    fg2 = load("fg2t", [128, 2], pool=work)
    fbe2 = load("fbe2t", [128, 2], pool=work)
    outT = big.tile([128, 2, 256], F32, tag="outT")
    for m in range(2):
        r2 = big.tile([128, 256], F32, tag="r2")
        nc.sync.dma_start(r2[:], d["ar6_o"][m * 128 : (m + 1) * 128, :])
        stf2 = work.tile([128, 6], F32, tag="stf2")
        nc.vector.bn_stats(stf2[:], r2[:])
        mvf2 = work.tile([128, 2], F32, tag="mvf2")
        nc.vector.bn_aggr(mvf2[:], stf2[:])
        nc.scalar.activation(mvf2[:, 1:2], mvf2[:, 1:2], AF.Sqrt, bias=epsf[:])
        nc.vector.reciprocal(mvf2[:, 1:2], mvf2[:, 1:2])
        scf2, bif2 = _scale_bias(nc, work, mvf2,
                                 fg2[:, m : m + 1], fbe2[:, m : m + 1], "f2")
        nc.scalar.activation(r2[:], r2[:], AF.Relu, bias=bif2[:], scale=scf2[:])
        for tt in range(2):
            ps_t = ps2.tile([128, 128], F32, tag="tailT")
            nc.tensor.transpose(ps_t[:], r2[:, tt * 128 : (tt + 1) * 128],
                                ident[:])
            nc.scalar.copy(outT[:, tt, m * 128 : (m + 1) * 128], ps_t[:])
    for tt in range(2):
        sq = big.tile([128, 256], F32, tag="sq")
        nc.scalar.activation(sq[:], outT[:, tt, :], AF.Square)
        nrm = work.tile([128, 1], F32, tag="nrm")
        nc.vector.reduce_sum(nrm[:], sq[:], axis=mybir.AxisListType.X)
        nc.scalar.activation(nrm[:], nrm[:], AF.Sqrt)
        nc.vector.tensor_scalar_max(nrm[:], nrm[:], 1e-12)
        nc.vector.reciprocal(nrm[:], nrm[:])
        nc.vector.tensor_scalar_mul(outT[:, tt, :], outT[:, tt, :], nrm[:])
        nc.sync.dma_start(d["out_final"][tt * 128 : (tt + 1) * 128, :],
                          outT[:, tt, :])

    ps2_cm.__exit__(None, None, None)
    work_cm.__exit__(None, None, None)
    big_cm.__exit__(None, None, None)
    sing_cm.__exit__(None, None, None)


# ------------------------------------------------------------------ host side
def _prep_core(x3, fw1, c):
    import ml_dtypes
    xs = x3[:, PL * c : PL * (c + 1), :]                       # [256,250,32]
    arr = np.ascontiguousarray(xs.transpose(2, 0, 1))          # [32,256,250]
    xA4 = arr.reshape(C, 4, QF).transpose(1, 0, 2).reshape(128, QF)
    xb = xs.reshape(B, 2, PH, C).transpose(2, 3, 1, 0)         # [125,32,2,256]
    xB = np.ascontiguousarray(xb).reshape(PH, C * 2 * B)
    fw = fw1.reshape(1024, P, C)[:, PL * c : PL * (c + 1), :]
    fw = fw.reshape(1024, 2, PH, C).transpose(2, 3, 1, 0)      # [125,32,2,1024]
    fw1t = np.ascontiguousarray(fw).reshape(PH, C * 2, 1024)
    bf = np.float16
    return (np.ascontiguousarray(xA4).astype(bf), xB.astype(bf),
            fw1t.astype(bf))


def _qrep(v, rows):
    out = np.zeros((128, 1), np.float32)
    for a in range(4):
        out[32 * a : 32 * a + rows, 0] = v
    return out


def _wdiag(w):
    """w [out,in] -> block-diagonal lhsT [128, 128]: block a (32x32) holds
    w.T in its top-left corner."""
    t = np.zeros((128, 128), np.float32)
    wt = w.T  # [in, out]
    for a in range(4):
        t[32 * a : 32 * a + wt.shape[0], 32 * a : 32 * a + wt.shape[1]] = wt
    return t


def kernel(**inputs):
    import ml_dtypes

    if "nc" not in _cache:
        _cache["nc"] = _build()
    nc = _cache["nc"]
    bf = np.float16

    g = {k: np.asarray(v, np.float32) for k, v in inputs.items()
         if k != "length"}
    x3 = g["x"].reshape(B, P, C)

    f1 = np.zeros((128, 16), np.float32)
    f2 = np.zeros((128, 8), np.float32)
    for a in range(4):
        f1[32 * a : 32 * a + 16, :] = np.eye(16, dtype=np.float32)
        f2[32 * a : 32 * a + 8, :] = np.eye(8, dtype=np.float32)
    f8_16 = np.zeros((128, 16), np.float32)
    f8_8 = np.zeros((64, 8), np.float32)
    for k in range(8):
        f8_16[16 * k : 16 * k + 16, :] = np.eye(16, dtype=np.float32)
        f8_8[8 * k : 8 * k + 8, :] = np.eye(8, dtype=np.float32)

    shared = {
        "w1D": _wdiag(g["w1"]).astype(bf),
        "w2D": _wdiag(g["w2"]).astype(bf),
        "w3D": _wdiag(g["w3"]).astype(bf),
        "g1q": _qrep(g["g1"], 16), "be1q": _qrep(g["be1"], 16),
        "g2q": _qrep(g["g2"], 8), "be2q": _qrep(g["be2"], 8),
        "g3s": g["g3"].reshape(1, 1), "be3s": g["be3"].reshape(1, 1),
        "f1": f1, "ft1": np.ascontiguousarray(f1.T),
        "f2": f2, "ft2": np.ascontiguousarray(f2.T),
        "f8_16": f8_16, "f8_8": f8_8,
        "fg2t": np.ascontiguousarray(g["fg2"].reshape(2, 128).T),
        "fbe2t": np.ascontiguousarray(g["fbe2"].reshape(2, 128).T),
    }

    in_maps = []
    for c in range(NCORES):
        xA4, xB, fw1t = _prep_core(x3, g["fw1"], c)
        m = dict(shared)
        m["xA4"] = xA4
        m["xB"] = xB
        m["fw1t"] = fw1t
        m["fw2t"] = np.ascontiguousarray(
            g["fw2"][:, 128 * c : 128 * (c + 1)].T).astype(bf)
        m["fg1s"] = g["fg1"][128 * c : 128 * (c + 1)].reshape(128, 1)
        m["fbe1s"] = g["fbe1"][128 * c : 128 * (c + 1)].reshape(128, 1)
        in_maps.append(m)

    from concourse.bass_utils import run_bass_kernel_spmd

    res = run_bass_kernel_spmd(nc, in_maps, core_ids=list(range(NCORES)),
                               trace=bool(_cache.get("trace")))
    _cache["last_result"] = res
    return np.asarray(res.results[0]["out_final"], np.float32)


if __name__ == "__main__":
    nc = _build()
    print("build ok; instructions:",
          sum(len(bb.instructions) for bb in nc.main_func.blocks))


# revision 21
# speedup vs baseline: 1.3138x; 1.0924x over previous
"""Trainium2 Bass kernel for nn_FCGF_point_att3_sft_7000 (8 NeuronCores).

Model: pointwise attention MLP (32->16->8->1, BN+relu, BN stats over the full
512000-point batch), per-segment softmax over 2000 points, attention-weighted
pooling to [256, 64000], FC head 64000->1024->256 (BN+relu, stats over the
256-segment batch), final L2 row-normalize.

Sharding: points-within-segment. Core c owns points p in [250c, 250(c+1)) of
every segment. Stage A is data-parallel over points with AllGather'd BN stats;
fc1 is contraction-sharded (each core owns 8000 of the 64000 inputs and the
matching fw1 rows), summed via ReduceScatter whose per-shard aux row also
carries the softmax denominators; fc2 is contraction-sharded and finished with
an AllReduce; the tail is replicated.

Stage-A layout: "quartered" A-orientation. x.T is [128, 16000] with the
channels of free-quarter a on partitions [32a, 32a+32). Matmuls use
tile_position=(32a, 32a) so outputs land on partitions 32a+ch and every
eviction / BN / softmax op runs 128 partitions wide. Weight tiles are
zero-padded to M=32 so all PSUM rows are defined.

Training-mode BN is shift-invariant => conv/linear biases (b1,b2,b3,fb1,fb2)
drop out exactly; they are accepted and ignored.
"""

import sys

sys.path.insert(0, "/opt/trn_rl_repo")

import numpy as np

import concourse.bass as bass
import concourse.tile as tile
from concourse import mybir
from concourse.masks import make_identity

B = 256
P = 2000
C = 32
NCORES = 8
PL = P // NCORES           # 250
PH = PL // 2               # 125
NPTS = B * PL              # 64000 points per core
QF = NPTS // 4             # 16000 per quarter
NCH = 500                  # stage-A free chunk
NCHUNK = QF // NCH         # 32
EPS_BN = 1e-5
F32 = mybir.dt.float32
BF16 = mybir.dt.float16  # fp16: same speed as bf16, 8x lower rounding noise
RG = [list(range(NCORES))]
AF = mybir.ActivationFunctionType

_cache = {}


# ------------------------------------------------------------------ walrus fix
def _install_walrus_patch():
    """This container's walrus accepts only ONE semaphore wait per instruction.
    Spread Tile's end-of-kernel drain waits across single-wait nops, and split
    any instruction carrying >1 waits onto same-engine carrier nops."""
    if _cache.get("patched"):
        return
    from concourse.vector_clock import ScopedClock, VectorClock

    counter = [0]

    def split_waits(nc):
        for bb in nc.main_func.blocks:
            out = []
            changed = False
            for ins in bb.instructions:
                si = ins.sync_info
                waits = list(si.on_wait) if si and si.on_wait else []
                if len(waits) > 1:
                    changed = True
                    for w in waits[:-1]:
                        counter[0] += 1
                        out.append(mybir.InstNoOp(
                            name=f"I-wsplit-{counter[0]}",
                            engine=ins.engine, ins=[], outs=[],
                            sync_info=mybir.SyncInfo(on_wait=[w], on_update=[]),
                            bass_nofuse=True))
                    si.on_wait = waits[-1:]
                out.append(ins)
            if changed:
                try:
                    bb.instructions = out
                except Exception:
                    bb.instructions.clear()
                    for x in out:
                        bb.instructions.append(x)

    def _patched(self, tick_clock, wait_clock):
        nc = self.nc
        gc = tick_clock.global_clock
        n = len(gc)
        for i in range(n):
            if gc[i] > 0:
                vec = [0] * n
                vec[i] = gc[i]
                nop = nc.sync.nop(nofuse=True, hint=f"drain_wait_p{i}")
                wait_clock.add_sem_waits(
                    nop.ins, ScopedClock({None: VectorClock(vec)}))
        nc.sync.drain()
        nc.all_engine_barrier()
        assert self.sems is not None
        popped = nc._tile_sem_poison_stack.pop()
        assert popped is self._sem_poison
        nc.clear_and_free_semaphores(list(self.sems.allocated().values()))
        nc.all_engine_barrier()
        split_waits(nc)

    tile.TileContext._drain_and_barrier = _patched
    _cache["patched"] = True


# ------------------------------------------------------------------ bass build
def _build():
    _install_walrus_patch()
    nc = bass.Bass()

    def ein(name, shape, dt):
        return nc.dram_tensor(name, shape, dt, kind="ExternalInput")

    d = {}
    d["xA4"] = ein("xA4", [128, QF], BF16)
    d["xB"] = ein("xB", [PH, C * 2 * B], BF16)
    d["w1D"] = ein("w1D", [128, 128], BF16)
    d["w2D"] = ein("w2D", [128, 128], BF16)
    d["w3D"] = ein("w3D", [128, 128], BF16)
    for n in ("g1q", "be1q", "g2q", "be2q"):
        d[n] = ein(n, [128, 1], F32)
    d["g3s"] = ein("g3s", [1, 1], F32)
    d["be3s"] = ein("be3s", [1, 1], F32)
    d["f1"] = ein("f1", [128, 16], F32)
    d["ft1"] = ein("ft1", [16, 128], F32)
    d["f2"] = ein("f2", [128, 8], F32)
    d["ft2"] = ein("ft2", [8, 128], F32)
    d["f8_16"] = ein("f8_16", [128, 16], F32)
    d["f8_8"] = ein("f8_8", [64, 8], F32)
    d["fw1t"] = ein("fw1t", [PH, C * 2, 1024], BF16)
    d["fw2t"] = ein("fw2t", [128, 256], BF16)
    d["fg1s"] = ein("fg1s", [128, 1], F32)
    d["fbe1s"] = ein("fbe1s", [128, 1], F32)
    d["fg2t"] = ein("fg2t", [128, 2], F32)
    d["fbe2t"] = ein("fbe2t", [128, 2], F32)
    d["out_final"] = nc.dram_tensor("out_final", [256, 256], F32,
                                    kind="ExternalOutput")
    # collective bounce buffers
    d["warm_i"] = nc.dram_tensor("warm_i", [16, 4], F32)
    d["warm_o"] = nc.dram_tensor("warm_o", [16, 4], F32)
    d["st1_i"] = nc.dram_tensor("st1_i", [16, 2], F32)
    d["st1_o"] = nc.dram_tensor("st1_o", [128, 2], F32)
    d["st2_i"] = nc.dram_tensor("st2_i", [8, 2], F32)
    d["st2_o"] = nc.dram_tensor("st2_o", [64, 2], F32)
    d["st3_i"] = nc.dram_tensor("st3_i", [1, 2], F32)
    d["st3_o"] = nc.dram_tensor("st3_o", [8, 2], F32)
    d["rs5_i"] = nc.dram_tensor("rs5_i", [NCORES * 129, 256], F32)
    d["rs5_o"] = nc.dram_tensor("rs5_o", [129, 256], F32)
    d["ar6_i"] = nc.dram_tensor("ar6_i", [256, 256], F32)
    d["ar6_o"] = nc.dram_tensor("ar6_o", [256, 256], F32)

    with tile.TileContext(nc) as tc:
        _body(nc, tc, d)
    return nc


def _mkstats(nc, pool, mv, count, name):
    """mv [p,2]=(mean,var) -> (sum,sumsq) [p,2]."""
    p = mv.shape[0]
    ss = pool.tile([p, 2], F32, tag=f"ss_{name}")
    nc.vector.tensor_mul(ss[:, 1:2], mv[:, 0:1], mv[:, 0:1])
    nc.vector.tensor_add(ss[:, 1:2], ss[:, 1:2], mv[:, 1:2])
    nc.scalar.mul(ss[:, 0:1], mv[:, 0:1], float(count))
    nc.scalar.mul(ss[:, 1:2], ss[:, 1:2], float(count))
    return ss


def _mv_from_ss(nc, pool, ss, count, name):
    """(sum,sumsq) [p,2] over count -> (mean, rstd) [p,2]."""
    p = ss.shape[0]
    mr = pool.tile([p, 2], F32, tag=f"mr_{name}")
    epst = pool.tile([p, 1], F32, tag=f"eps_{name}")
    nc.vector.memset(epst[:], EPS_BN)
    nc.scalar.mul(mr[:, 0:1], ss[:, 0:1], 1.0 / count)
    nc.scalar.mul(mr[:, 1:2], ss[:, 1:2], 1.0 / count)
    m2 = pool.tile([p, 1], F32, tag=f"m2_{name}")
    nc.vector.tensor_mul(m2[:], mr[:, 0:1], mr[:, 0:1])
    nc.vector.tensor_sub(mr[:, 1:2], mr[:, 1:2], m2[:])
    nc.scalar.activation(mr[:, 1:2], mr[:, 1:2], AF.Sqrt, bias=epst[:])
    nc.vector.reciprocal(mr[:, 1:2], mr[:, 1:2])
    return mr


def _scale_bias(nc, pool, mrq, g, be, name):
    """scale = g*rstd ; bias = be - scale*mean  (all [p,1] per-partition)."""
    p = mrq.shape[0]
    sc = pool.tile([p, 1], F32, tag=f"sc_{name}")
    bi = pool.tile([p, 1], F32, tag=f"bi_{name}")
    nc.vector.tensor_mul(sc[:], g[:], mrq[:, 1:2])
    nc.vector.tensor_mul(bi[:], sc[:], mrq[:, 0:1])
    nc.vector.tensor_sub(bi[:], be[:], bi[:])
    return sc, bi


def _body(nc, tc, d):
    # collective warmup first — input copied dram->dram (no engine deps), so
    # the ~55us ncfw startup overlaps the whole front of the kernel.
    nc.gpsimd.dma_start(d["warm_i"][:], d["f1"][0:16, 0:4])
    nc.gpsimd.collective_compute(
        "AllReduce", mybir.AluOpType.add, replica_groups=RG,
        ins=[d["warm_i"][:]], outs=[d["warm_o"][:]])
    nc.gpsimd.collective_compute(
        "AllGather", mybir.AluOpType.bypass, replica_groups=RG,
        ins=[d["warm_i"][0:2, :]], outs=[d["warm_o"][0:16, :]])

    sing_cm = tc.tile_pool(name="sing", bufs=1)
    big_cm = tc.tile_pool(name="big", bufs=1)
    work_cm = tc.tile_pool(name="work", bufs=1)
    psA_cm = tc.tile_pool(name="psA", bufs=4, space="PSUM")
    psT_cm = tc.tile_pool(name="psT", bufs=2, space="PSUM")
    psS_cm = tc.tile_pool(name="psS", bufs=2, space="PSUM")
    sing = sing_cm.__enter__(); big = big_cm.__enter__()
    work = work_cm.__enter__()
    fw1p_cm = tc.tile_pool(name="fw1p", bufs=2)
    fw1p = fw1p_cm.__enter__()
    psA = psA_cm.__enter__(); psT = psT_cm.__enter__()
    psS = psS_cm.__enter__()

    # ---------------- constants
    def load(name, shape, dt=F32, pool=sing):
        t = pool.tile(shape, dt, tag=name)
        nc.sync.dma_start(t[:], d[name][:])
        return t

    w1D = load("w1D", [128, 128], BF16)
    w2D = load("w2D", [128, 128], BF16)
    w3D = load("w3D", [128, 128], BF16)
    f1s = load("f1", [128, 16])
    ft1s = load("ft1", [16, 128])
    f2s = load("f2", [128, 8])
    ft2s = load("ft2", [8, 128])
    f8_16s = load("f8_16", [128, 16])
    f8_8s = load("f8_8", [64, 8])
    g1 = load("g1q", [128, 1]); be1 = load("be1q", [128, 1])
    g2 = load("g2q", [128, 1]); be2 = load("be2q", [128, 1])
    g3 = load("g3s", [1, 1]); be3 = load("be3s", [1, 1])
    ones128 = sing.tile([128, 1], F32)
    nc.vector.memset(ones128[:], 1.0)
    ones8 = sing.tile([8, 1], F32)
    nc.vector.memset(ones8[:], 1.0)
    ones1x = sing.tile([1, 128], F32)
    nc.vector.memset(ones1x[:], 1.0)
    ident = sing.tile([128, 128], F32)
    make_identity(nc, ident[:])

    # ---------------- big loads
    xa = big.tile([128, QF], BF16, tag="slotA")       # slot A: xa -> y2 -> y3q
    nc.sync.dma_start(xa[:], d["xA4"][:])
    xb = big.tile([PH, C * 2 * B], BF16, tag="xb")
    nc.sync.dma_start(xb[:], d["xB"][:])
    xbv = xb[:].rearrange("p (c h s) -> p c h s", c=C, h=2, s=B)

    # fc1 weight prefetch: pool entered at the top so its slots exist from
    # t=0 and the 16.4MB stream overlaps all of stage A. 3-engine rotation.
    ITS_PER_DMA = 16
    fwview = d["fw1t"][:].rearrange("p (g i) o -> p g i o", i=ITS_PER_DMA)
    fwtiles = []
    _dge = [nc.sync, nc.scalar, nc.gpsimd]
    for gblk in range(C * 2 // ITS_PER_DMA):
        fwt = fw1p.tile([PH, ITS_PER_DMA, 1024], BF16, tag="fw",
                        name=f"fw_{gblk}")
        _dge[gblk % 3].dma_start(fwt[:], fwview[:, gblk, :, :])
        fwtiles.append(fwt)

    def layer_mms(ps, wD, krows, rhs_src, sl):
        nc.tensor.matmul(ps[:], wD[:], rhs_src[:, sl], start=True, stop=True)

    def stage_layer(rhs_src, wT, krows, fold, foldT, f8fold, st_i, st_o,
                    gq, beq, count_local, name, out_tag):
        """Single-pass layer: matmuls -> evict y f32 (+bn_stats from PSUM),
        fold+AllGather stats, then BN+relu applied in place (h aliases y)."""
        y = big.tile([128, QF], BF16, tag=out_tag, name=f"y_{name}")
        stat = work.tile([128, NCHUNK, 6], F32, tag=f"stat_{name}")
        for j in range(NCHUNK):
            ps = psA.tile([128, NCH], F32, tag="psA", name=f"ps_{name}_{j}")
            layer_mms(ps, wT, krows, rhs_src, slice(j * NCH, (j + 1) * NCH))
            nc.scalar.copy(y[:, j * NCH : (j + 1) * NCH], ps[:])
            nc.vector.bn_stats(stat[:, j, :], ps[:])
        mv = work.tile([128, 2], F32, tag=f"mv_{name}")
        nc.vector.bn_aggr(mv[:], stat[:])
        ss = _mkstats(nc, work, mv, count_local, name)
        nfold = fold.shape[1]
        psf = psS.tile([128, 2], F32, tag="small", name=f"psf_{name}")
        nc.tensor.matmul(psf[:nfold, :], fold[:], ss[:], start=True, stop=True)
        sbf = work.tile([nfold, 2], F32, tag=f"sbf_{name}")
        nc.scalar.copy(sbf[:], psf[:nfold, :])
        nc.gpsimd.dma_start(st_i[:], sbf[:])
        nc.gpsimd.collective_compute(
            "AllGather", mybir.AluOpType.bypass, replica_groups=RG,
            ins=[st_i[:]], outs=[st_o[:]])
        agg = work.tile([nfold * NCORES, 2], F32, tag=f"agg_{name}")
        nc.gpsimd.dma_start(agg[:], st_o[:])
        psg = psS.tile([128, 2], F32, tag="small", name=f"psg_{name}")
        nc.tensor.matmul(psg[:nfold, :], f8fold[:], agg[:], start=True,
                         stop=True)
        ssg = work.tile([nfold, 2], F32, tag=f"ssg_{name}")
        nc.scalar.copy(ssg[:], psg[:nfold, :])
        mr = _mv_from_ss(nc, work, ssg, B * P, name)
        psb = psS.tile([128, 2], F32, tag="small", name=f"psb_{name}")
        nc.tensor.matmul(psb[:], foldT[:], mr[:], start=True, stop=True)
        mrq = work.tile([128, 2], F32, tag=f"mrq_{name}")
        nc.scalar.copy(mrq[:], psb[:])
        sc, bi = _scale_bias(nc, work, mrq, gq, beq, name)
        for j in range(NCHUNK):
            sl = slice(j * NCH, (j + 1) * NCH)
            nc.scalar.activation(y[:, sl], y[:, sl], AF.Relu,
                                 bias=bi[:], scale=sc[:])
        return y

    # ---------------- stage A layers 1 & 2
    h1 = stage_layer(xa, w1D, 32, f1s, ft1s, f8_16s,
                     d["st1_i"], d["st1_o"], g1, be1, QF, "l1", "slotB")
    # h2 reuses slot A (xa dead after L1 matmuls)
    h2 = stage_layer(h1, w2D, 16, f2s, ft2s, f8_8s,
                     d["st2_i"], d["st2_o"], g2, be2, QF, "l2", "slotA")

    # ---------------- stage A layer 3: scores straight from PSUM into
    # scoreS [128 segs, 2, 250] via per-chunk repack DMAs (rows {32a} real;
    # chunk j of quarter a covers segments 64a+2j..+1)
    y3q = big.tile([128, QF], BF16, tag="slotB", name="y3q")
    for j in range(NCHUNK):
        ps = psA.tile([128, NCH], F32, tag="psA", name=f"ps_l3_{j}")
        layer_mms(ps, w3D, 8, h2, slice(j * NCH, (j + 1) * NCH))
        nc.scalar.copy(y3q[:, j * NCH : (j + 1) * NCH], ps[:])
    scoreS = big.tile([128, 2, PL], BF16, tag="scoreS")
    for a in range(4):
        nc.sync.dma_start(
            scoreS[64 * (a % 2) : 64 * (a % 2) + 64, a // 2, :],
            y3q[32 * a : 32 * a + 1, :])
    # BN3 stats over all segments/points (all partitions real)
    stat3 = work.tile([128, 2, 6], F32, tag="stat3")
    nc.vector.bn_stats(stat3[:, 0, :], scoreS[:, 0, :])
    nc.vector.bn_stats(stat3[:, 1, :], scoreS[:, 1, :])
    mv3 = work.tile([128, 2], F32, tag="mv3")
    nc.vector.bn_aggr(mv3[:], stat3[:])
    ss3 = _mkstats(nc, work, mv3, 2 * PL, "l3")
    psf3 = psS.tile([128, 2], F32, tag="small", name="psf3")
    nc.tensor.matmul(psf3[:1, :], ones128[:], ss3[:], start=True, stop=True)
    sbf3 = work.tile([1, 2], F32, tag="sbf3")
    nc.scalar.copy(sbf3[:], psf3[:1, :])
    nc.gpsimd.dma_start(d["st3_i"][:], sbf3[:])
    nc.gpsimd.collective_compute(
        "AllGather", mybir.AluOpType.bypass, replica_groups=RG,
        ins=[d["st3_i"][:]], outs=[d["st3_o"][:]])
    agg3 = work.tile([8, 2], F32, tag="agg3")
    nc.gpsimd.dma_start(agg3[:], d["st3_o"][:])
    psg3 = psS.tile([128, 2], F32, tag="small", name="psg3")
    nc.tensor.matmul(psg3[:1, :], ones8[:], agg3[:], start=True, stop=True)
    ssg3 = work.tile([1, 2], F32, tag="ssg3")
    nc.scalar.copy(ssg3[:], psg3[:1, :])
    mr3 = _mv_from_ss(nc, work, ssg3, B * P, "l3")
    scb1 = work.tile([1, 2], F32, tag="scb1")
    nc.vector.tensor_mul(scb1[:, 0:1], g3[:], mr3[:, 1:2])
    nc.vector.tensor_mul(scb1[:, 1:2], scb1[:, 0:1], mr3[:, 0:1])
    nc.vector.tensor_sub(scb1[:, 1:2], be3[:], scb1[:, 1:2])
    psb3 = psS.tile([128, 2], F32, tag="small", name="psb3")
    nc.tensor.matmul(psb3[:], ones1x[:], scb1[:], start=True, stop=True)
    scb = work.tile([128, 2], F32, tag="scb")
    nc.scalar.copy(scb[:], psb3[:])
    # relu(BN3) in place, then exp
    expS = big.tile([128, 2, PL], F32, tag="expS")
    for tt in range(2):
        nc.scalar.activation(scoreS[:, tt, :], scoreS[:, tt, :], AF.Relu,
                             bias=scb[:, 1:2], scale=scb[:, 0:1])
        nc.scalar.activation(expS[:, tt, :], scoreS[:, tt, :], AF.Exp)
        # partial softmax denominators
    zloc = work.tile([128, 2], F32, tag="zloc")
    nc.vector.reduce_sum(zloc[:, 0:1], expS[:, 0, :], axis=mybir.AxisListType.X)
    nc.vector.reduce_sum(zloc[:, 1:2], expS[:, 1, :], axis=mybir.AxisListType.X)
    # z into every shard's aux row of rs5_i (8 small DMAs; dst col = 128*tt+s)
    for cc in range(NCORES):
        dst = d["rs5_i"][cc * 129 + 128 : cc * 129 + 129, :].rearrange(
            "r (t s) -> r s t", t=2, s=128)
        nc.sync.dma_start(dst, zloc[:])
    # expT [125, 2, 256]: PE-transpose expS halves
    expT = big.tile([PH, 2, 256], F32, tag="expT")
    for h in range(2):
        for tt in range(2):
            pt_ps = psT.tile([128, 128], F32, tag="psT")
            nc.tensor.transpose(pt_ps[:PH, :],
                                expS[:, tt, h * PH : h * PH + PH], ident[:])
            nc.scalar.copy(expT[:, h, tt * 128 : tt * 128 + 128],
                           pt_ps[:PH, :])

    psS_cm.__exit__(None, None, None)
    psT_cm.__exit__(None, None, None)
    psA_cm.__exit__(None, None, None)

    # ---------------- FC1 (contraction-sharded, out [1024, 256] partial)
    psF_cm = tc.tile_pool(name="psF", bufs=1, space="PSUM")
    ptp_cm = tc.tile_pool(name="ptp", bufs=3)
    psF = psF_cm.__enter__()
    ptp = ptp_cm.__enter__()
    r1ps = [psF.tile([128, 256], F32, name=f"r1ps_{m}", tag=f"r1_{m}")
            for m in range(8)]
    NIT = C * 2
    for ch in range(C):
        for h in range(2):
            it = ch * 2 + h
            fw = fwtiles[it // ITS_PER_DMA][:, it % ITS_PER_DMA, :]
            pt = ptp.tile([PH, 256], BF16, tag="pt", name=f"pt_{it}")
            nc.vector.tensor_mul(pt[:], xbv[:, ch, h, :], expT[:, h, :])
            for m in range(8):
                nc.tensor.matmul(
                    r1ps[m][:, :], fw[:, m * 128 : (m + 1) * 128], pt[:],
                    start=(it == 0), stop=(it == NIT - 1))
    for m in range(8):
        r1sb = big.tile([128, 256], F32, tag="r1sb", name=f"r1sb_{m}", bufs=2)
        nc.scalar.copy(r1sb[:], r1ps[m][:])
        nc.sync.dma_start(d["rs5_i"][m * 129 : m * 129 + 128, :], r1sb[:])
    nc.gpsimd.collective_compute(
        "ReduceScatter", mybir.AluOpType.add, replica_groups=RG,
        ins=[d["rs5_i"][:]], outs=[d["rs5_o"][:]])

    ptp_cm.__exit__(None, None, None)
    psF_cm.__exit__(None, None, None)
    fw1p_cm.__exit__(None, None, None)

    # ---------------- FC1 finish + FC2 + tail
    ps2_cm = tc.tile_pool(name="ps2", bufs=1, space="PSUM")
    ps2 = ps2_cm.__enter__()

    r1 = big.tile([128, 256], F32, tag="r1")
    nc.sync.dma_start(r1[:], d["rs5_o"][0:128, :])
    zrow = work.tile([1, 256], F32, tag="zrow")
    nc.sync.dma_start(zrow[:], d["rs5_o"][128:129, :])
    nc.vector.reciprocal(zrow[:], zrow[:])
    ps_z = ps2.tile([128, 256], F32, tag="zb")
    nc.tensor.matmul(ps_z[:], ones1x[:], zrow[:], start=True, stop=True)
    zinv = big.tile([128, 256], F32, tag="zinv")
    nc.scalar.copy(zinv[:], ps_z[:])
    nc.vector.tensor_mul(r1[:], r1[:], zinv[:])
    # BN over segments (free dim), relu
    stf1 = work.tile([128, 6], F32, tag="stf1")
    nc.vector.bn_stats(stf1[:], r1[:])
    mvf1 = work.tile([128, 2], F32, tag="mvf1")
    nc.vector.bn_aggr(mvf1[:], stf1[:])
    epsf = work.tile([128, 1], F32, tag="epsf")
    nc.vector.memset(epsf[:], EPS_BN)
    nc.scalar.activation(mvf1[:, 1:2], mvf1[:, 1:2], AF.Sqrt, bias=epsf[:])
    nc.vector.reciprocal(mvf1[:, 1:2], mvf1[:, 1:2])
    fg1 = load("fg1s", [128, 1], pool=work)
    fbe1 = load("fbe1s", [128, 1], pool=work)
    scf1, bif1 = _scale_bias(nc, work, mvf1, fg1, fbe1, "f1")
    r1b = big.tile([128, 256], BF16, tag="r1b")
    nc.scalar.activation(r1b[:], r1[:], AF.Relu, bias=bif1[:], scale=scf1[:])
    # FC2 partial
    fw2 = load("fw2t", [128, 256], BF16, pool=work)
    r2sb = big.tile([128, 2, 256], F32, tag="r2sb")
    for m in range(2):
        ps_r2 = ps2.tile([128, 256], F32, tag=f"r2_{m}")
        nc.tensor.matmul(ps_r2[:], fw2[:, m * 128 : (m + 1) * 128], r1b[:],
                         start=True, stop=True)
        nc.scalar.copy(r2sb[:, m, :], ps_r2[:])
        nc.sync.dma_start(d["ar6_i"][m * 128 : (m + 1) * 128, :],
                          r2sb[:, m, :])
    nc.gpsimd.collective_compute(
        "AllReduce", mybir.AluOpType.add, replica_groups=RG,
        ins=[d["ar6_i"][:]], outs=[d["ar6_o"][:]])

    # tail: BN over segments per o2-row, relu, transpose, L2-normalize
    fg2 = load("fg2t", [128, 2], pool=work)
    fbe2 = load("fbe2t", [128, 2], pool=work)
    outT = big.tile([128, 2, 256], F32, tag="outT")
    for m in range(2):
        r2 = big.tile([128, 256], F32, tag="r2")
        nc.sync.dma_start(r2[:], d["ar6_o"][m * 128 : (m + 1) * 128, :])
        stf2 = work.tile([128, 6], F32, tag="stf2")
        nc.vector.bn_stats(stf2[:], r2[:])
        mvf2 = work.tile([128, 2], F32, tag="mvf2")
        nc.vector.bn_aggr(mvf2[:], stf2[:])
        nc.scalar.activation(mvf2[:, 1:2], mvf2[:, 1:2], AF.Sqrt, bias=epsf[:])
        nc.vector.reciprocal(mvf2[:, 1:2], mvf2[:, 1:2])
        scf2, bif2 = _scale_bias(nc, work, mvf2,
                                 fg2[:, m : m + 1], fbe2[:, m : m + 1], "f2")
        nc.scalar.activation(r2[:], r2[:], AF.Relu, bias=bif2[:], scale=scf2[:])
        for tt in range(2):
            ps_t = ps2.tile([128, 128], F32, tag="tailT")
            nc.tensor.transpose(ps_t[:], r2[:, tt * 128 : (tt + 1) * 128],
                                ident[:])
            nc.scalar.copy(outT[:, tt, m * 128 : (m + 1) * 128], ps_t[:])
    for tt in range(2):
        sq = big.tile([128, 256], F32, tag="sq")
        nc.scalar.activation(sq[:], outT[:, tt, :], AF.Square)
        nrm = work.tile([128, 1], F32, tag="nrm")
        nc.vector.reduce_sum(nrm[:], sq[:], axis=mybir.AxisListType.X)
        nc.scalar.activation(nrm[:], nrm[:], AF.Sqrt)
        nc.vector.tensor_scalar_max(nrm[:], nrm[:], 1e-12)
        nc.vector.reciprocal(nrm[:], nrm[:])
        nc.vector.tensor_scalar_mul(outT[:, tt, :], outT[:, tt, :], nrm[:])
        nc.sync.dma_start(d["out_final"][tt * 128 : (tt + 1) * 128, :],
                          outT[:, tt, :])

    ps2_cm.__exit__(None, None, None)
    work_cm.__exit__(None, None, None)
    big_cm.__exit__(None, None, None)
    sing_cm.__exit__(None, None, None)


# ------------------------------------------------------------------ host side
def _prep_core(x3, fw1, c):
    import ml_dtypes
    xs = x3[:, PL * c : PL * (c + 1), :]                       # [256,250,32]
    arr = np.ascontiguousarray(xs.transpose(2, 0, 1))          # [32,256,250]
    xA4 = arr.reshape(C, 4, QF).transpose(1, 0, 2).reshape(128, QF)
    xb = xs.reshape(B, 2, PH, C).transpose(2, 3, 1, 0)         # [125,32,2,256]
    xB = np.ascontiguousarray(xb).reshape(PH, C * 2 * B)
    fw = fw1.reshape(1024, P, C)[:, PL * c : PL * (c + 1), :]
    fw = fw.reshape(1024, 2, PH, C).transpose(2, 3, 1, 0)      # [125,32,2,1024]
    fw1t = np.ascontiguousarray(fw).reshape(PH, C * 2, 1024)
    bf = np.float16
    return (np.ascontiguousarray(xA4).astype(bf), xB.astype(bf),
            fw1t.astype(bf))


def _qrep(v, rows):
    out = np.zeros((128, 1), np.float32)
    for a in range(4):
        out[32 * a : 32 * a + rows, 0] = v
    return out


def _wdiag(w):
    """w [out,in] -> block-diagonal lhsT [128, 128]: block a (32x32) holds
    w.T in its top-left corner."""
    t = np.zeros((128, 128), np.float32)
    wt = w.T  # [in, out]
    for a in range(4):
        t[32 * a : 32 * a + wt.shape[0], 32 * a : 32 * a + wt.shape[1]] = wt
    return t


def kernel(**inputs):
    import ml_dtypes

    if "nc" not in _cache:
        _cache["nc"] = _build()
    nc = _cache["nc"]
    bf = np.float16

    g = {k: np.asarray(v, np.float32) for k, v in inputs.items()
         if k != "length"}
    x3 = g["x"].reshape(B, P, C)

    f1 = np.zeros((128, 16), np.float32)
    f2 = np.zeros((128, 8), np.float32)
    for a in range(4):
        f1[32 * a : 32 * a + 16, :] = np.eye(16, dtype=np.float32)
        f2[32 * a : 32 * a + 8, :] = np.eye(8, dtype=np.float32)
    f8_16 = np.zeros((128, 16), np.float32)
    f8_8 = np.zeros((64, 8), np.float32)
    for k in range(8):
        f8_16[16 * k : 16 * k + 16, :] = np.eye(16, dtype=np.float32)
        f8_8[8 * k : 8 * k + 8, :] = np.eye(8, dtype=np.float32)

    shared = {
        "w1D": _wdiag(g["w1"]).astype(bf),
        "w2D": _wdiag(g["w2"]).astype(bf),
        "w3D": _wdiag(g["w3"]).astype(bf),
        "g1q": _qrep(g["g1"], 16), "be1q": _qrep(g["be1"], 16),
        "g2q": _qrep(g["g2"], 8), "be2q": _qrep(g["be2"], 8),
        "g3s": g["g3"].reshape(1, 1), "be3s": g["be3"].reshape(1, 1),
        "f1": f1, "ft1": np.ascontiguousarray(f1.T),
        "f2": f2, "ft2": np.ascontiguousarray(f2.T),
        "f8_16": f8_16, "f8_8": f8_8,
        "fg2t": np.ascontiguousarray(g["fg2"].reshape(2, 128).T),
        "fbe2t": np.ascontiguousarray(g["fbe2"].reshape(2, 128).T),
    }

    in_maps = []
    for c in range(NCORES):
        xA4, xB, fw1t = _prep_core(x3, g["fw1"], c)
        m = dict(shared)
        m["xA4"] = xA4
        m["xB"] = xB
        m["fw1t"] = fw1t
        m["fw2t"] = np.ascontiguousarray(
            g["fw2"][:, 128 * c : 128 * (c + 1)].T).astype(bf)
        m["fg1s"] = g["fg1"][128 * c : 128 * (c + 1)].reshape(128, 1)
        m["fbe1s"] = g["fbe1"][128 * c : 128 * (c + 1)].reshape(128, 1)
        in_maps.append(m)

    from concourse.bass_utils import run_bass_kernel_spmd

    res = run_bass_kernel_spmd(nc, in_maps, core_ids=list(range(NCORES)),
                               trace=bool(_cache.get("trace")))
    _cache["last_result"] = res
    return np.asarray(res.results[0]["out_final"], np.float32)


if __name__ == "__main__":
    nc = _build()
    print("build ok; instructions:",
          sum(len(bb.instructions) for bb in nc.main_func.blocks))


# revision 23
# speedup vs baseline: 1.4358x; 1.0928x over previous
"""Trainium2 Bass kernel for nn_FCGF_point_att3_sft_7000 (8 NeuronCores).

Model: pointwise attention MLP (32->16->8->1, BN+relu, BN stats over the full
512000-point batch), per-segment softmax over 2000 points, attention-weighted
pooling to [256, 64000], FC head 64000->1024->256 (BN+relu, stats over the
256-segment batch), final L2 row-normalize.

Sharding: points-within-segment. Core c owns points p in [250c, 250(c+1)) of
every segment. Stage A is data-parallel over points with AllGather'd BN stats;
fc1 is contraction-sharded (each core owns 8000 of the 64000 inputs and the
matching fw1 rows), summed via ReduceScatter whose per-shard aux row also
carries the softmax denominators; fc2 is contraction-sharded and finished with
an AllReduce; the tail is replicated.

Stage-A layout: "quartered" A-orientation. x.T is [128, 16000] with the
channels of free-quarter a on partitions [32a, 32a+32). Matmuls use
tile_position=(32a, 32a) so outputs land on partitions 32a+ch and every
eviction / BN / softmax op runs 128 partitions wide. Weight tiles are
zero-padded to M=32 so all PSUM rows are defined.

Training-mode BN is shift-invariant => conv/linear biases (b1,b2,b3,fb1,fb2)
drop out exactly; they are accepted and ignored.
"""

import sys

sys.path.insert(0, "/opt/trn_rl_repo")

import numpy as np

import concourse.bass as bass
import concourse.tile as tile
from concourse import mybir
from concourse.masks import make_identity

B = 256
P = 2000
C = 32
NCORES = 8
PL = P // NCORES           # 250
PH = PL // 2               # 125
NPTS = B * PL              # 64000 points per core
QF = NPTS // 4             # 16000 per quarter
NCH = 500                  # stage-A free chunk
NCHUNK = QF // NCH         # 32
EPS_BN = 1e-5
F32 = mybir.dt.float32
BF16 = mybir.dt.float16  # fp16: same speed as bf16, 8x lower rounding noise
RG = [list(range(NCORES))]
AF = mybir.ActivationFunctionType

_cache = {}


# ------------------------------------------------------------------ walrus fix
def _install_walrus_patch():
    """This container's walrus accepts only ONE semaphore wait per instruction.
    Spread Tile's end-of-kernel drain waits across single-wait nops, and split
    any instruction carrying >1 waits onto same-engine carrier nops."""
    if _cache.get("patched"):
        return
    from concourse.vector_clock import ScopedClock, VectorClock

    counter = [0]

    def split_waits(nc):
        for bb in nc.main_func.blocks:
            out = []
            changed = False
            for ins in bb.instructions:
                si = ins.sync_info
                waits = list(si.on_wait) if si and si.on_wait else []
                if len(waits) > 1:
                    changed = True
                    for w in waits[:-1]:
                        counter[0] += 1
                        out.append(mybir.InstNoOp(
                            name=f"I-wsplit-{counter[0]}",
                            engine=ins.engine, ins=[], outs=[],
                            sync_info=mybir.SyncInfo(on_wait=[w], on_update=[]),
                            bass_nofuse=True))
                    si.on_wait = waits[-1:]
                out.append(ins)
            if changed:
                try:
                    bb.instructions = out
                except Exception:
                    bb.instructions.clear()
                    for x in out:
                        bb.instructions.append(x)

    def _patched(self, tick_clock, wait_clock):
        nc = self.nc
        gc = tick_clock.global_clock
        n = len(gc)
        for i in range(n):
            if gc[i] > 0:
                vec = [0] * n
                vec[i] = gc[i]
                nop = nc.sync.nop(nofuse=True, hint=f"drain_wait_p{i}")
                wait_clock.add_sem_waits(
                    nop.ins, ScopedClock({None: VectorClock(vec)}))
        nc.sync.drain()
        nc.all_engine_barrier()
        assert self.sems is not None
        popped = nc._tile_sem_poison_stack.pop()
        assert popped is self._sem_poison
        nc.clear_and_free_semaphores(list(self.sems.allocated().values()))
        nc.all_engine_barrier()
        split_waits(nc)

    tile.TileContext._drain_and_barrier = _patched
    _cache["patched"] = True


# ------------------------------------------------------------------ bass build
def _build():
    _install_walrus_patch()
    nc = bass.Bass()

    def ein(name, shape, dt):
        return nc.dram_tensor(name, shape, dt, kind="ExternalInput")

    d = {}
    d["xA4"] = ein("xA4", [128, QF], BF16)
    d["xB"] = ein("xB", [PH, C * 2 * B], BF16)
    d["w1D"] = ein("w1D", [128, 128], BF16)
    d["w2D"] = ein("w2D", [128, 128], BF16)
    d["w3D"] = ein("w3D", [128, 128], BF16)
    for n in ("g1q", "be1q", "g2q", "be2q"):
        d[n] = ein(n, [128, 1], F32)
    d["g3s"] = ein("g3s", [1, 1], F32)
    d["be3s"] = ein("be3s", [1, 1], F32)
    d["f1"] = ein("f1", [128, 16], F32)
    d["ft1"] = ein("ft1", [16, 128], F32)
    d["f2"] = ein("f2", [128, 8], F32)
    d["ft2"] = ein("ft2", [8, 128], F32)
    d["f8_16"] = ein("f8_16", [128, 16], F32)
    d["f8_8"] = ein("f8_8", [64, 8], F32)
    d["fw1t"] = ein("fw1t", [PH, C * 2, 1024], BF16)
    d["fw2t"] = ein("fw2t", [128, 256], BF16)
    d["fg1s"] = ein("fg1s", [128, 1], F32)
    d["fbe1s"] = ein("fbe1s", [128, 1], F32)
    d["fg2t"] = ein("fg2t", [128, 2], F32)
    d["fbe2t"] = ein("fbe2t", [128, 2], F32)
    d["out_final"] = nc.dram_tensor("out_final", [256, 256], F32,
                                    kind="ExternalOutput")
    # collective bounce buffers
    d["warm_i"] = nc.dram_tensor("warm_i", [16, 4], F32)
    d["warm_o"] = nc.dram_tensor("warm_o", [16, 4], F32)
    d["st1_i"] = nc.dram_tensor("st1_i", [16, 2], F32)
    d["st1_o"] = nc.dram_tensor("st1_o", [128, 2], F32)
    d["st2_i"] = nc.dram_tensor("st2_i", [8, 2], F32)
    d["st2_o"] = nc.dram_tensor("st2_o", [64, 2], F32)
    d["st3_i"] = nc.dram_tensor("st3_i", [1, 2], F32)
    d["st3_o"] = nc.dram_tensor("st3_o", [8, 2], F32)
    d["rs5_i"] = nc.dram_tensor("rs5_i", [NCORES * 129, 256], F32)
    d["rs5_o"] = nc.dram_tensor("rs5_o", [129, 256], F32)
    d["ar6_i"] = nc.dram_tensor("ar6_i", [256, 256], F32)
    d["ar6_o"] = nc.dram_tensor("ar6_o", [256, 256], F32)

    with tile.TileContext(nc) as tc:
        _body(nc, tc, d)
    return nc


def _mkstats(nc, pool, mv, count, name):
    """mv [p,2]=(mean,var) -> (sum,sumsq) [p,2]."""
    p = mv.shape[0]
    ss = pool.tile([p, 2], F32, tag=f"ss_{name}")
    nc.vector.tensor_mul(ss[:, 1:2], mv[:, 0:1], mv[:, 0:1])
    nc.vector.tensor_add(ss[:, 1:2], ss[:, 1:2], mv[:, 1:2])
    nc.scalar.mul(ss[:, 0:1], mv[:, 0:1], float(count))
    nc.scalar.mul(ss[:, 1:2], ss[:, 1:2], float(count))
    return ss


def _mv_from_ss(nc, pool, ss, count, name):
    """(sum,sumsq) [p,2] over count -> (mean, rstd) [p,2]."""
    p = ss.shape[0]
    mr = pool.tile([p, 2], F32, tag=f"mr_{name}")
    epst = pool.tile([p, 1], F32, tag=f"eps_{name}")
    nc.vector.memset(epst[:], EPS_BN)
    nc.scalar.mul(mr[:, 0:1], ss[:, 0:1], 1.0 / count)
    nc.scalar.mul(mr[:, 1:2], ss[:, 1:2], 1.0 / count)
    m2 = pool.tile([p, 1], F32, tag=f"m2_{name}")
    nc.vector.tensor_mul(m2[:], mr[:, 0:1], mr[:, 0:1])
    nc.vector.tensor_sub(mr[:, 1:2], mr[:, 1:2], m2[:])
    nc.scalar.activation(mr[:, 1:2], mr[:, 1:2], AF.Sqrt, bias=epst[:])
    nc.vector.reciprocal(mr[:, 1:2], mr[:, 1:2])
    return mr


def _scale_bias(nc, pool, mrq, g, be, name):
    """scale = g*rstd ; bias = be - scale*mean  (all [p,1] per-partition)."""
    p = mrq.shape[0]
    sc = pool.tile([p, 1], F32, tag=f"sc_{name}")
    bi = pool.tile([p, 1], F32, tag=f"bi_{name}")
    nc.vector.tensor_mul(sc[:], g[:], mrq[:, 1:2])
    nc.vector.tensor_mul(bi[:], sc[:], mrq[:, 0:1])
    nc.vector.tensor_sub(bi[:], be[:], bi[:])
    return sc, bi


def _body(nc, tc, d):
    # collective warmup first — input copied dram->dram (no engine deps), so
    # the ~55us ncfw startup overlaps the whole front of the kernel.
    nc.gpsimd.dma_start(d["warm_i"][:], d["f1"][0:16, 0:4])
    nc.gpsimd.collective_compute(
        "AllReduce", mybir.AluOpType.add, replica_groups=RG,
        ins=[d["warm_i"][:]], outs=[d["warm_o"][:]])
    sing_cm = tc.tile_pool(name="sing", bufs=1)
    big_cm = tc.tile_pool(name="big", bufs=1)
    work_cm = tc.tile_pool(name="work", bufs=1)
    psA_cm = tc.tile_pool(name="psA", bufs=4, space="PSUM")
    psT_cm = tc.tile_pool(name="psT", bufs=2, space="PSUM")
    psS_cm = tc.tile_pool(name="psS", bufs=2, space="PSUM")
    sing = sing_cm.__enter__(); big = big_cm.__enter__()
    work = work_cm.__enter__()
    fw1p_cm = tc.tile_pool(name="fw1p", bufs=2)
    fw1p = fw1p_cm.__enter__()
    psA = psA_cm.__enter__(); psT = psT_cm.__enter__()
    psS = psS_cm.__enter__()

    # ---------------- constants
    def load(name, shape, dt=F32, pool=sing):
        t = pool.tile(shape, dt, tag=name)
        nc.sync.dma_start(t[:], d[name][:])
        return t

    w1D = load("w1D", [128, 128], BF16)
    w2D = load("w2D", [128, 128], BF16)
    w3D = load("w3D", [128, 128], BF16)
    f1s = load("f1", [128, 16])
    ft1s = load("ft1", [16, 128])
    f2s = load("f2", [128, 8])
    ft2s = load("ft2", [8, 128])
    f8_16s = load("f8_16", [128, 16])
    f8_8s = load("f8_8", [64, 8])
    g1 = load("g1q", [128, 1]); be1 = load("be1q", [128, 1])
    g2 = load("g2q", [128, 1]); be2 = load("be2q", [128, 1])
    g3 = load("g3s", [1, 1]); be3 = load("be3s", [1, 1])
    ones128 = sing.tile([128, 1], F32)
    nc.vector.memset(ones128[:], 1.0)
    ones8 = sing.tile([8, 1], F32)
    nc.vector.memset(ones8[:], 1.0)
    ones1x = sing.tile([1, 128], F32)
    nc.vector.memset(ones1x[:], 1.0)
    ident = sing.tile([128, 128], F32)
    make_identity(nc, ident[:])

    # ---------------- big loads
    xa = big.tile([128, QF], BF16, tag="slotA")       # slot A: xa -> y2 -> y3q
    nc.sync.dma_start(xa[:], d["xA4"][:])
    xb = big.tile([PH, C * 2 * B], BF16, tag="xb")
    nc.sync.dma_start(xb[:], d["xB"][:])
    xbv = xb[:].rearrange("p (c h s) -> p c h s", c=C, h=2, s=B)

    # fc1 weight prefetch: pool entered at the top so its slots exist from
    # t=0 and the 16.4MB stream overlaps all of stage A. 3-engine rotation.
    FW_CHUNKS = [22, 21, 21]
    fwtiles = []
    _dge = [nc.sync, nc.scalar]
    _off = 0
    for gblk, nits in enumerate(FW_CHUNKS):
        fwt = fw1p.tile([PH, 22, 1024], BF16, tag="fw", name=f"fw_{gblk}")
        _dge[gblk % 2].dma_start(fwt[:, :nits, :],
                                 d["fw1t"][:, _off : _off + nits, :])
        fwtiles.append((fwt, _off, nits))
        _off += nits

    def layer_mms(ps, wD, krows, rhs_src, sl):
        nc.tensor.matmul(ps[:], wD[:], rhs_src[:, sl], start=True, stop=True)

    def stage_layer(rhs_src, wT, krows, fold, foldT, f8fold, st_i, st_o,
                    gq, beq, count_local, name, out_tag):
        """Single-pass layer: matmuls -> evict y f32 (+bn_stats from PSUM),
        fold+AllGather stats, then BN+relu applied in place (h aliases y)."""
        y = big.tile([128, QF], BF16, tag=out_tag, name=f"y_{name}")
        stat = work.tile([128, NCHUNK, 6], F32, tag=f"stat_{name}")
        for j in range(NCHUNK):
            ps = psA.tile([128, NCH], F32, tag="psA", name=f"ps_{name}_{j}")
            layer_mms(ps, wT, krows, rhs_src, slice(j * NCH, (j + 1) * NCH))
            nc.scalar.copy(y[:, j * NCH : (j + 1) * NCH], ps[:])
            nc.vector.bn_stats(stat[:, j, :], ps[:])
        mv = work.tile([128, 2], F32, tag=f"mv_{name}")
        nc.vector.bn_aggr(mv[:], stat[:])
        ss = _mkstats(nc, work, mv, count_local, name)
        nfold = fold.shape[1]
        psf = psS.tile([128, 2], F32, tag="small", name=f"psf_{name}")
        nc.tensor.matmul(psf[:nfold, :], fold[:], ss[:], start=True, stop=True)
        sbf = work.tile([nfold, 2], F32, tag=f"sbf_{name}")
        nc.scalar.copy(sbf[:], psf[:nfold, :])
        nc.gpsimd.dma_start(st_i[:], sbf[:])
        nc.gpsimd.collective_compute(
            "AllGather", mybir.AluOpType.bypass, replica_groups=RG,
            ins=[st_i[:]], outs=[st_o[:]])
        agg = work.tile([nfold * NCORES, 2], F32, tag=f"agg_{name}")
        nc.gpsimd.dma_start(agg[:], st_o[:])
        psg = psS.tile([128, 2], F32, tag="small", name=f"psg_{name}")
        nc.tensor.matmul(psg[:nfold, :], f8fold[:], agg[:], start=True,
                         stop=True)
        ssg = work.tile([nfold, 2], F32, tag=f"ssg_{name}")
        nc.scalar.copy(ssg[:], psg[:nfold, :])
        mr = _mv_from_ss(nc, work, ssg, B * P, name)
        psb = psS.tile([128, 2], F32, tag="small", name=f"psb_{name}")
        nc.tensor.matmul(psb[:], foldT[:], mr[:], start=True, stop=True)
        mrq = work.tile([128, 2], F32, tag=f"mrq_{name}")
        nc.scalar.copy(mrq[:], psb[:])
        sc, bi = _scale_bias(nc, work, mrq, gq, beq, name)
        for j in range(NCHUNK):
            sl = slice(j * NCH, (j + 1) * NCH)
            nc.scalar.activation(y[:, sl], y[:, sl], AF.Relu,
                                 bias=bi[:], scale=sc[:])
        return y

    # ---------------- stage A layers 1 & 2
    h1 = stage_layer(xa, w1D, 32, f1s, ft1s, f8_16s,
                     d["st1_i"], d["st1_o"], g1, be1, QF, "l1", "slotB")
    # h2 reuses slot A (xa dead after L1 matmuls)
    h2 = stage_layer(h1, w2D, 16, f2s, ft2s, f8_8s,
                     d["st2_i"], d["st2_o"], g2, be2, QF, "l2", "slotA")

    # ---------------- stage A layer 3: scores straight from PSUM into
    # scoreS [128 segs, 2, 250] via per-chunk repack DMAs (rows {32a} real;
    # chunk j of quarter a covers segments 64a+2j..+1)
    y3q = big.tile([128, QF], BF16, tag="slotB", name="y3q")
    for j in range(NCHUNK):
        ps = psA.tile([128, NCH], F32, tag="psA", name=f"ps_l3_{j}")
        layer_mms(ps, w3D, 8, h2, slice(j * NCH, (j + 1) * NCH))
        nc.scalar.copy(y3q[:, j * NCH : (j + 1) * NCH], ps[:])
    scoreS = big.tile([128, 2, PL], BF16, tag="scoreS")
    for a in range(4):
        nc.sync.dma_start(
            scoreS[64 * (a % 2) : 64 * (a % 2) + 64, a // 2, :],
            y3q[32 * a : 32 * a + 1, :])
    # BN3 stats over all segments/points (all partitions real)
    stat3 = work.tile([128, 2, 6], F32, tag="stat3")
    nc.vector.bn_stats(stat3[:, 0, :], scoreS[:, 0, :])
    nc.vector.bn_stats(stat3[:, 1, :], scoreS[:, 1, :])
    mv3 = work.tile([128, 2], F32, tag="mv3")
    nc.vector.bn_aggr(mv3[:], stat3[:])
    ss3 = _mkstats(nc, work, mv3, 2 * PL, "l3")
    psf3 = psS.tile([128, 2], F32, tag="small", name="psf3")
    nc.tensor.matmul(psf3[:1, :], ones128[:], ss3[:], start=True, stop=True)
    sbf3 = work.tile([1, 2], F32, tag="sbf3")
    nc.scalar.copy(sbf3[:], psf3[:1, :])
    nc.gpsimd.dma_start(d["st3_i"][:], sbf3[:])
    nc.gpsimd.collective_compute(
        "AllGather", mybir.AluOpType.bypass, replica_groups=RG,
        ins=[d["st3_i"][:]], outs=[d["st3_o"][:]])
    agg3 = work.tile([8, 2], F32, tag="agg3")
    nc.gpsimd.dma_start(agg3[:], d["st3_o"][:])
    psg3 = psS.tile([128, 2], F32, tag="small", name="psg3")
    nc.tensor.matmul(psg3[:1, :], ones8[:], agg3[:], start=True, stop=True)
    ssg3 = work.tile([1, 2], F32, tag="ssg3")
    nc.scalar.copy(ssg3[:], psg3[:1, :])
    mr3 = _mv_from_ss(nc, work, ssg3, B * P, "l3")
    scb1 = work.tile([1, 2], F32, tag="scb1")
    nc.vector.tensor_mul(scb1[:, 0:1], g3[:], mr3[:, 1:2])
    nc.vector.tensor_mul(scb1[:, 1:2], scb1[:, 0:1], mr3[:, 0:1])
    nc.vector.tensor_sub(scb1[:, 1:2], be3[:], scb1[:, 1:2])
    psb3 = psS.tile([128, 2], F32, tag="small", name="psb3")
    nc.tensor.matmul(psb3[:], ones1x[:], scb1[:], start=True, stop=True)
    scb = work.tile([128, 2], F32, tag="scb")
    nc.scalar.copy(scb[:], psb3[:])
    # relu(BN3) in place, then exp
    expS = big.tile([128, 2, PL], F32, tag="expS")
    for tt in range(2):
        nc.scalar.activation(scoreS[:, tt, :], scoreS[:, tt, :], AF.Relu,
                             bias=scb[:, 1:2], scale=scb[:, 0:1])
        nc.scalar.activation(expS[:, tt, :], scoreS[:, tt, :], AF.Exp)
        # partial softmax denominators
    zloc = work.tile([128, 2], F32, tag="zloc")
    nc.vector.reduce_sum(zloc[:, 0:1], expS[:, 0, :], axis=mybir.AxisListType.X)
    nc.vector.reduce_sum(zloc[:, 1:2], expS[:, 1, :], axis=mybir.AxisListType.X)
    # z into every shard's aux row of rs5_i (8 small DMAs; dst col = 128*tt+s)
    for cc in range(NCORES):
        dst = d["rs5_i"][cc * 129 + 128 : cc * 129 + 129, :].rearrange(
            "r (t s) -> r s t", t=2, s=128)
        nc.sync.dma_start(dst, zloc[:])
    # expT [125, 2, 256]: PE-transpose expS halves
    expT = big.tile([PH, 2, 256], F32, tag="expT")
    for h in range(2):
        for tt in range(2):
            pt_ps = psT.tile([128, 128], F32, tag="psT")
            nc.tensor.transpose(pt_ps[:PH, :],
                                expS[:, tt, h * PH : h * PH + PH], ident[:])
            nc.scalar.copy(expT[:, h, tt * 128 : tt * 128 + 128],
                           pt_ps[:PH, :])

    psS_cm.__exit__(None, None, None)
    psT_cm.__exit__(None, None, None)
    psA_cm.__exit__(None, None, None)

    # ---------------- FC1 (contraction-sharded, out [1024, 256] partial)
    psF_cm = tc.tile_pool(name="psF", bufs=1, space="PSUM")
    ptp_cm = tc.tile_pool(name="ptp", bufs=3)
    psF = psF_cm.__enter__()
    ptp = ptp_cm.__enter__()
    r1ps = [psF.tile([128, 256], F32, name=f"r1ps_{m}", tag=f"r1_{m}")
            for m in range(8)]
    NIT = C * 2
    for ch in range(C):
        for h in range(2):
            it = ch * 2 + h
            gi = 0
            while not (fwtiles[gi][1] <= it < fwtiles[gi][1] + fwtiles[gi][2]):
                gi += 1
            fw = fwtiles[gi][0][:, it - fwtiles[gi][1], :]
            pt = ptp.tile([PH, 256], BF16, tag="pt", name=f"pt_{it}")
            nc.vector.tensor_mul(pt[:], xbv[:, ch, h, :], expT[:, h, :])
            for m in range(8):
                nc.tensor.matmul(
                    r1ps[m][:, :], fw[:, m * 128 : (m + 1) * 128], pt[:],
                    start=(it == 0), stop=(it == NIT - 1))
    for m in range(8):
        r1sb = big.tile([128, 256], F32, tag="r1sb", name=f"r1sb_{m}", bufs=2)
        nc.scalar.copy(r1sb[:], r1ps[m][:])
        nc.sync.dma_start(d["rs5_i"][m * 129 : m * 129 + 128, :], r1sb[:])
    nc.gpsimd.collective_compute(
        "ReduceScatter", mybir.AluOpType.add, replica_groups=RG,
        ins=[d["rs5_i"][:]], outs=[d["rs5_o"][:]])

    ptp_cm.__exit__(None, None, None)
    psF_cm.__exit__(None, None, None)
    fw1p_cm.__exit__(None, None, None)

    # ---------------- FC1 finish + FC2 + tail
    ps2_cm = tc.tile_pool(name="ps2", bufs=1, space="PSUM")
    ps2 = ps2_cm.__enter__()

    r1 = big.tile([128, 256], F32, tag="r1")
    nc.sync.dma_start(r1[:], d["rs5_o"][0:128, :])
    zrow = work.tile([1, 256], F32, tag="zrow")
    nc.sync.dma_start(zrow[:], d["rs5_o"][128:129, :])
    nc.vector.reciprocal(zrow[:], zrow[:])
    ps_z = ps2.tile([128, 256], F32, tag="zb")
    nc.tensor.matmul(ps_z[:], ones1x[:], zrow[:], start=True, stop=True)
    zinv = big.tile([128, 256], F32, tag="zinv")
    nc.scalar.copy(zinv[:], ps_z[:])
    nc.vector.tensor_mul(r1[:], r1[:], zinv[:])
    # BN over segments (free dim), relu
    stf1 = work.tile([128, 6], F32, tag="stf1")
    nc.vector.bn_stats(stf1[:], r1[:])
    mvf1 = work.tile([128, 2], F32, tag="mvf1")
    nc.vector.bn_aggr(mvf1[:], stf1[:])
    epsf = work.tile([128, 1], F32, tag="epsf")
    nc.vector.memset(epsf[:], EPS_BN)
    nc.scalar.activation(mvf1[:, 1:2], mvf1[:, 1:2], AF.Sqrt, bias=epsf[:])
    nc.vector.reciprocal(mvf1[:, 1:2], mvf1[:, 1:2])
    fg1 = load("fg1s", [128, 1], pool=work)
    fbe1 = load("fbe1s", [128, 1], pool=work)
    scf1, bif1 = _scale_bias(nc, work, mvf1, fg1, fbe1, "f1")
    r1b = big.tile([128, 256], BF16, tag="r1b")
    nc.scalar.activation(r1b[:], r1[:], AF.Relu, bias=bif1[:], scale=scf1[:])
    # FC2 partial
    fw2 = load("fw2t", [128, 256], BF16, pool=work)
    r2sb = big.tile([128, 2, 256], F32, tag="r2sb")
    for m in range(2):
        ps_r2 = ps2.tile([128, 256], F32, tag=f"r2_{m}")
        nc.tensor.matmul(ps_r2[:], fw2[:, m * 128 : (m + 1) * 128], r1b[:],
                         start=True, stop=True)
        nc.scalar.copy(r2sb[:, m, :], ps_r2[:])
        nc.sync.dma_start(d["ar6_i"][m * 128 : (m + 1) * 128, :],
                          r2sb[:, m, :])
    nc.gpsimd.collective_compute(
        "AllReduce", mybir.AluOpType.add, replica_groups=RG,
        ins=[d["ar6_i"][:]], outs=[d["ar6_o"][:]])

    # tail: BN over segments per o2-row, relu, transpose, L2-normalize
    fg2 = load("fg2t", [128, 2], pool=work)
    fbe2 = load("fbe2t", [128, 2], pool=work)
    outT = big.tile([128, 2, 256], F32, tag="outT")
    for m in range(2):
        r2 = big.tile([128, 256], F32, tag="r2")
        nc.sync.dma_start(r2[:], d["ar6_o"][m * 128 : (m + 1) * 128, :])
        stf2 = work.tile([128, 6], F32, tag="stf2")
        nc.vector.bn_stats(stf2[:], r2[:])
        mvf2 = work.tile([128, 2], F32, tag="mvf2")
        nc.vector.bn_aggr(mvf2[:], stf2[:])
        nc.scalar.activation(mvf2[:, 1:2], mvf2[:, 1:2], AF.Sqrt, bias=epsf[:])
        nc.vector.reciprocal(mvf2[:, 1:2], mvf2[:, 1:2])
        scf2, bif2 = _scale_bias(nc, work, mvf2,
                                 fg2[:, m : m + 1], fbe2[:, m : m + 1], "f2")
        nc.scalar.activation(r2[:], r2[:], AF.Relu, bias=bif2[:], scale=scf2[:])
        for tt in range(2):
            ps_t = ps2.tile([128, 128], F32, tag="tailT")
            nc.tensor.transpose(ps_t[:], r2[:, tt * 128 : (tt + 1) * 128],
                                ident[:])
            nc.scalar.copy(outT[:, tt, m * 128 : (m + 1) * 128], ps_t[:])
    for tt in range(2):
        sq = big.tile([128, 256], F32, tag="sq")
        nc.scalar.activation(sq[:], outT[:, tt, :], AF.Square)
        nrm = work.tile([128, 1], F32, tag="nrm")
        nc.vector.reduce_sum(nrm[:], sq[:], axis=mybir.AxisListType.X)
        nc.scalar.activation(nrm[:], nrm[:], AF.Sqrt)
        nc.vector.tensor_scalar_max(nrm[:], nrm[:], 1e-12)
        nc.vector.reciprocal(nrm[:], nrm[:])
        nc.vector.tensor_scalar_mul(outT[:, tt, :], outT[:, tt, :], nrm[:])
        nc.sync.dma_start(d["out_final"][tt * 128 : (tt + 1) * 128, :],
                          outT[:, tt, :])

    ps2_cm.__exit__(None, None, None)
    work_cm.__exit__(None, None, None)
    big_cm.__exit__(None, None, None)
    sing_cm.__exit__(None, None, None)


# ------------------------------------------------------------------ host side
def _prep_core(x3, fw1, c):
    import ml_dtypes
    xs = x3[:, PL * c : PL * (c + 1), :]                       # [256,250,32]
    arr = np.ascontiguousarray(xs.transpose(2, 0, 1))          # [32,256,250]
    xA4 = arr.reshape(C, 4, QF).transpose(1, 0, 2).reshape(128, QF)
    xb = xs.reshape(B, 2, PH, C).transpose(2, 3, 1, 0)         # [125,32,2,256]
    xB = np.ascontiguousarray(xb).reshape(PH, C * 2 * B)
    fw = fw1.reshape(1024, P, C)[:, PL * c : PL * (c + 1), :]
    fw = fw.reshape(1024, 2, PH, C).transpose(2, 3, 1, 0)      # [125,32,2,1024]
    fw1t = np.ascontiguousarray(fw).reshape(PH, C * 2, 1024)
    bf = np.float16
    return (np.ascontiguousarray(xA4).astype(bf), xB.astype(bf),
            fw1t.astype(bf))


def _qrep(v, rows):
    out = np.zeros((128, 1), np.float32)
    for a in range(4):
        out[32 * a : 32 * a + rows, 0] = v
    return out


def _wdiag(w):
    """w [out,in] -> block-diagonal lhsT [128, 128]: block a (32x32) holds
    w.T in its top-left corner."""
    t = np.zeros((128, 128), np.float32)
    wt = w.T  # [in, out]
    for a in range(4):
        t[32 * a : 32 * a + wt.shape[0], 32 * a : 32 * a + wt.shape[1]] = wt
    return t


def kernel(**inputs):
    import ml_dtypes

    if "nc" not in _cache:
        _cache["nc"] = _build()
    nc = _cache["nc"]
    bf = np.float16

    g = {k: np.asarray(v, np.float32) for k, v in inputs.items()
         if k != "length"}
    x3 = g["x"].reshape(B, P, C)

    f1 = np.zeros((128, 16), np.float32)
    f2 = np.zeros((128, 8), np.float32)
    for a in range(4):
        f1[32 * a : 32 * a + 16, :] = np.eye(16, dtype=np.float32)
        f2[32 * a : 32 * a + 8, :] = np.eye(8, dtype=np.float32)
    f8_16 = np.zeros((128, 16), np.float32)
    f8_8 = np.zeros((64, 8), np.float32)
    for k in range(8):
        f8_16[16 * k : 16 * k + 16, :] = np.eye(16, dtype=np.float32)
        f8_8[8 * k : 8 * k + 8, :] = np.eye(8, dtype=np.float32)

    shared = {
        "w1D": _wdiag(g["w1"]).astype(bf),
        "w2D": _wdiag(g["w2"]).astype(bf),
        "w3D": _wdiag(g["w3"]).astype(bf),
        "g1q": _qrep(g["g1"], 16), "be1q": _qrep(g["be1"], 16),
        "g2q": _qrep(g["g2"], 8), "be2q": _qrep(g["be2"], 8),
        "g3s": g["g3"].reshape(1, 1), "be3s": g["be3"].reshape(1, 1),
        "f1": f1, "ft1": np.ascontiguousarray(f1.T),
        "f2": f2, "ft2": np.ascontiguousarray(f2.T),
        "f8_16": f8_16, "f8_8": f8_8,
        "fg2t": np.ascontiguousarray(g["fg2"].reshape(2, 128).T),
        "fbe2t": np.ascontiguousarray(g["fbe2"].reshape(2, 128).T),
    }

    in_maps = []
    for c in range(NCORES):
        xA4, xB, fw1t = _prep_core(x3, g["fw1"], c)
        m = dict(shared)
        m["xA4"] = xA4
        m["xB"] = xB
        m["fw1t"] = fw1t
        m["fw2t"] = np.ascontiguousarray(
            g["fw2"][:, 128 * c : 128 * (c + 1)].T).astype(bf)
        m["fg1s"] = g["fg1"][128 * c : 128 * (c + 1)].reshape(128, 1)
        m["fbe1s"] = g["fbe1"][128 * c : 128 * (c + 1)].reshape(128, 1)
        in_maps.append(m)

    from concourse.bass_utils import run_bass_kernel_spmd

    res = run_bass_kernel_spmd(nc, in_maps, core_ids=list(range(NCORES)),
                               trace=bool(_cache.get("trace")))
    _cache["last_result"] = res
    return np.asarray(res.results[0]["out_final"], np.float32)


if __name__ == "__main__":
    nc = _build()
    print("build ok; instructions:",
          sum(len(bb.instructions) for bb in nc.main_func.blocks))
